# revision 80
# baseline (speedup 1.0000x reference)
"""Dual-stream linear-attention transformer — bf16 redesign (per-core).

Layout convention (same as baseline):
  - "layout 1" activation: [E, N] feature-major; SBUF tiles [128, KE, C]
    (feature f = 128*k + p -> partition p, k-th slice; tokens on free dim).
  - alpha k/v are produced token-major per 128-token tile [128, E].
  - Residual streams live in internal DRAM as [E, N] bf16.

Key design points:
  - All matmul operands + SBUF activations bf16; biases folded into
    matmuls (rank-1 ones_row accumulates) or Act per-partition bias.
  - QKV down+up projections composed into single E x E / E x 2E weights
    on the host.
  - Every Act function stays inside ONE activation-table set
    (natural_log_exp_and_others): LayerNorm rsqrt = exp(-0.5*ln(v+eps)),
    gating sigmoid = 1/(1+exp(-x)) via DVE reciprocal, and
    _steer_act_tables biases the table-selection pass so exactly one
    LoadActFuncSet is emitted (the greedy pass otherwise thrashes
    ~84-150 reloads x 1.3us between the exp- and ln-only sets).
  - elu+1 split Act/Pool/DVE: Act Exp + Act Relu, Pool min(.,1), DVE add.
  - FFN dwconv taps are BN-A-folded on the host; per-m conv legs are
    stage-major and split Act (w0 tap via per-partition-scale Copy),
    DVE (center tap, adds, halos, relu6 min) and Pool (w2 tap mul).
  - LN tails: residual add via Act Identity(bias)+DVE TT, squares
    emitted per slice so the stats matmuls start early.
  - Scheduling is deliberately phase-DENSE (no fine interleaving): the
    TimelineSim PE pstate model runs matmuls at 2.4GHz only after ~3us
    of continuous PE activity (788ns cold / 427ns warm / 213ns hot per
    512-col bf16 matmul), so long unbroken matmul bursts beat any
    work-spreading weave. Cross-phase overlap via DRAM store->load
    roundtrips always lost: an inserted stage that is not instantly
    ready head-of-line blocks the in-order PE queue.
  - Beta-side attention weights (wq/ow) load during late alpha chunks;
    hardware limits honored: Pool has no scalar_tensor_tensor, DVE
    tensor ops may read at most one PSUM operand.
"""

from dataclasses import dataclass
from contextlib import ExitStack

import numpy as np

import concourse.bass as bass
import concourse.mybir as mybir
import concourse.tile as tile

F32 = mybir.dt.float32
BF16 = mybir.dt.bfloat16
AF = mybir.ActivationFunctionType
ALU = mybir.AluOpType

LN_EPS = 1e-5
BN_EPS = 1e-5


@dataclass
class Cfg:
    N: int = 2048
    E: int = 512
    R: int = 256
    X: int = 1024
    H: int = 8
    L: int = 3
    OUT: int = 15
    C: int = 512

    @property
    def KE(self):
        return self.E // 128

    @property
    def KX(self):
        return self.X // 128

    @property
    def NC(self):
        return self.N // self.C

    @property
    def NTT(self):
        return self.C // 128


PHASES = []


def build(nc, cfg):
    c = cfg
    E, X, H, N, C, L = c.E, c.X, c.H, c.N, c.C, c.L
    KE, KX, NC, NTT = c.KE, c.KX, c.NC, c.NTT
    E4, E2, E8 = E // 4, E // 2, E // 8

    din = {}

    def inp(name, shape, dt=BF16):
        din[name] = nc.dram_tensor(name, list(shape), dt, kind="ExternalInput")
        return din[name].ap()

    # activations (host converts to bf16 and pre-transposes to [E, N])
    body_feats = inp("body_feats", (E, N))
    limb_feats = inp("limb_feats", (E, N))
    # attention weights (host-composed)
    wq = inp("wq", (L, 4, E, E))
    wkv = inp("wkv", (L, 4, E, 2 * E))
    ubq = inp("ubq", (L, 4, E), F32)
    ubkv = inp("ubkv", (L, 4, 2 * E))
    ow = inp("ow", (L, 4, E, E))
    obf = inp("ob", (L, 4, E), F32)
    # FFN
    w1 = inp("w1", (L, 2, E, X))
    b1f = inp("b1", (L, 2, X), F32)
    cwf = inp("cwf", (L, 2, 3, X), F32)     # conv taps, tap-major, BN-A folded
    bnB = inp("bnB", (L, 2, X), F32)        # cb*A + bnb
    w2 = inp("w2", (L, 2, X, E))
    b2f = inp("b2", (L, 2, E), F32)
    lng = inp("lng", (L, 5, E), F32)
    lnb = inp("lnb", (L, 5, E), F32)
    # gating
    gw1 = inp("gw1", (L, 2 * E, E4))
    gb1 = inp("gb1", (L, E4), F32)
    gwd = inp("gwd", (L, E4))               # gw2[:,0]-gw2[:,1]
    gb2d = inp("gb2d", (L, 1), F32)         # gb2[0]-gb2[1]
    # final head
    fw1 = inp("fw1", (2 * E, E2))
    fb1 = inp("fb1", (E2,), F32)
    fw2 = inp("fw2", (E2, E))
    fb2f = inp("fb2", (E,), F32)
    flng = inp("flng", (E,), F32)
    flnb = inp("flnb", (E,), F32)
    rw1 = inp("rw1", (E, E4))
    rb1 = inp("rb1", (E4,), F32)
    rw2 = inp("rw2", (E4, E8))
    rb2 = inp("rb2", (E8,), F32)
    rw3p = inp("rw3p", (E8, 16))            # zero-padded to 16
    rb3p = inp("rb3p", (16,))               # zero-padded
    ident_in = inp("ident", (128, 128))
    ones_in = inp("ones128", (128, 128))
    hmask_in = inp("hmask", (E, H))
    cmask_in = inp("cmask", (H, E))

    out_dram = nc.dram_tensor("out", [N, c.OUT], F32, kind="ExternalOutput")

    def idram(name):
        return nc.dram_tensor(name, [E, N], BF16).ap().rearrange(
            "(k p) n -> p k n", p=128)

    rs = {}
    for s in ("b", "l"):
        for l in range(L):
            for st in (1, 2, 3):
                rs[s, (l, st)] = idram(f"r{s}_{l}_{st}")
    rs["b", 0] = body_feats.rearrange("(k p) n -> p k n", p=128)
    rs["l", 0] = limb_feats.rearrange("(k p) n -> p k n", p=128)

    lowp = nc.allow_low_precision("bf16 activations within rel-err budget")

    with tile.TileContext(nc) as tc, ExitStack() as ctx, lowp:
        p_ = ctx.enter_context
        cst = p_(tc.tile_pool(name="cst", bufs=1))
        wbig = p_(tc.tile_pool(name="wbig", bufs=3))
        wsm = p_(tc.tile_pool(name="wsm", bufs=2))
        wcol = p_(tc.tile_pool(name="wcol", bufs=10))
        wrow = p_(tc.tile_pool(name="wrow", bufs=4))
        pa = p_(tc.tile_pool(name="pa", bufs=10))      # 4KB bf16 chunk tiles
        pb = p_(tc.tile_pool(name="pb", bufs=5))      # 8KB ht tiles
        pc = p_(tc.tile_pool(name="pc", bufs=12))      # 1KB bf16 / rows
        pat = p_(tc.tile_pool(name="pat", bufs=3))    # per-attn persistents
        phl = p_(tc.tile_pool(name="phl", bufs=6))    # conv halos
        ps = p_(tc.tile_pool(name="ps", bufs=6, space="PSUM"))
        psr = p_(tc.tile_pool(name="psr", bufs=2, space="PSUM"))

        v, sc, gp = nc.vector, nc.scalar, nc.gpsimd

        def mm(out, lhsT, rhs, start, stop):
            nc.tensor.matmul(out, lhsT, rhs, start=start, stop=stop)

        # ---- constants (ones first: the alpha bias matmuls need it;
        # hmask/cmask/ident aren't read until the bd/beta stages, so their
        # loads queue behind the first weight + activation chunks) ----
        ones_t = cst.tile([128, 128], BF16, tag="ones")
        nc.sync.dma_start(out=ones_t, in_=ones_in)
        def _late_consts():
            ident_t = cst.tile([128, 128], BF16, tag="ident")
            nc.sync.dma_start(out=ident_t, in_=ident_in)
            hmask_t = cst.tile([128, KE, H], BF16, tag="hmask")
            nc.sync.dma_start(out=hmask_t,
                              in_=hmask_in.rearrange("(k p) h -> p k h",
                                                     p=128))
            cmask_t = cst.tile([H, KE, 128], BF16, tag="cmask")
            nc.sync.dma_start(out=cmask_t,
                              in_=cmask_in.rearrange("h (k p) -> h k p",
                                                     p=128))
            return hmask_t, cmask_t
        ONES_COL = ones_t[:, 0:1]
        ONES_ROW = ones_t[0:1, :]
        onesc_t = cst.tile([1, C], BF16, tag="onesc")
        v.memset(onesc_t, 1.0)
        ONES_C = onesc_t[0:1, :]
        eps_ln = cst.tile([1, 1], F32, tag="epsl")
        v.memset(eps_ln, LN_EPS)

        def col_tile(src_ap, m, tag="col", bufs=None):
            t = wcol.tile([128, m], F32, tag=tag,
                          bufs=(12 if tag == "col" else bufs))
            nc.sync.dma_start(out=t, in_=src_ap.rearrange("(m p) -> p m", p=128))
            return t

        def row_tile(src_ap, n, tag="row", pool=None):
            t = (pool or wrow).tile([1, n], BF16, tag=tag)
            nc.sync.dma_start(out=t, in_=src_ap[None, :])
            return t

        def ln_apply(xs, g_col, b_col, outt, sq=None):
            """LayerNorm over features (layout 1). xs: [128, KE, C] bf16 tile.
            outt: [128, KE, C] bf16 out. rsqrt via exp(-0.5*ln(v+eps)) keeps
            every Act func in one table set (no LoadActFuncSet thrash).
            Callers that produce xs slice-by-slice pass a precomputed sq so
            the stats matmuls start after the first slice, not the last."""
            if sq is None:
                sq = pa.tile([128, KE, C], BF16, tag="a4")
                xf = xs.rearrange("p k c -> p (k c)")
                v.tensor_tensor(out=sq.rearrange("p k c -> p (k c)"),
                                in0=xf, in1=xf, op=ALU.mult)
            ps_s = psr.tile([1, C], F32, tag="row")
            ps_ss = psr.tile([1, C], F32, tag="row")
            for m in range(KE):
                mm(ps_s, ONES_COL, xs[:, m, :], start=(m == 0),
                   stop=(m == KE - 1))
                mm(ps_ss, ONES_COL, sq[:, m, :], start=(m == 0),
                   stop=(m == KE - 1))
            mrow = pc.tile([1, C], F32, tag="row", bufs=8)
            v.tensor_scalar_mul(mrow, ps_s, 1.0 / E)
            m2 = pc.tile([1, C], F32, tag="row", bufs=8)
            v.tensor_tensor(out=m2, in0=mrow, in1=mrow, op=ALU.mult)
            var = pc.tile([1, C], F32, tag="row", bufs=8)
            v.scalar_tensor_tensor(out=var, in0=ps_ss, scalar=1.0 / E,
                                   in1=m2, op0=ALU.mult, op1=ALU.subtract)
            lnv = pc.tile([1, C], F32, tag="row", bufs=8)
            sc.activation(lnv, var, AF.Ln, bias=eps_ln[0:1, 0:1])
            srow = pc.tile([1, C], BF16, tag="rowh", bufs=6)
            sc.activation(srow, lnv, AF.Exp, scale=-0.5)
            trow = pc.tile([1, C], BF16, tag="rowh", bufs=6)
            v.scalar_tensor_tensor(out=trow, in0=ps_s, scalar=1.0 / E,
                                   in1=srow, op0=ALU.mult, op1=ALU.mult)
            sb_s = pc.tile([128, C], BF16, tag="a1")
            gp.partition_broadcast(sb_s, srow)
            sb_t = pc.tile([128, C], BF16, tag="a1")
            gp.partition_broadcast(sb_t, trow)
            for m in range(KE):
                u = pc.tile([128, C], BF16, tag="a1")
                v.tensor_tensor(out=u, in0=xs[:, m, :], in1=sb_s, op=ALU.mult)
                v.tensor_tensor(out=u, in0=u, in1=sb_t, op=ALU.subtract)
                v.tensor_scalar(out=outt[:, m, :], in0=u,
                                scalar1=g_col[:, m:m + 1],
                                scalar2=b_col[:, m:m + 1],
                                op0=ALU.mult, op1=ALU.add)

        def load_x_chunk(dram_l1, ci, tag="a4"):
            xt = pa.tile([128, KE, C], BF16, tag=tag)
            nc.sync.dma_start(out=xt, in_=dram_l1[:, :, ci * C:(ci + 1) * C])
            return xt

        def store_chunk(dram_l1, ci, t):
            gp.dma_start(out=dram_l1[:, :, ci * C:(ci + 1) * C], in_=t)


        # ---- linear attention ----
        def attn_gen(l, a, xq_dram, xkv_dram, tail_m, tail_post,
                     prefetch=True):
            """Generator stages: wkv loads | a1/a2 per chunk | bd (+ wq/ow
            loads) | b1/b2a/b2b/b3 per chunk. With prefetch=False the alpha
            x-chunk loads are issued only at their own stage, so a load
            never waits at the DMA queue head for a producer store that
            has not happened yet (used when interleaved with the producing
            FFN)."""
            wkvt = wbig.tile([128, KE, 2 * E], BF16, tag="w2x", bufs=4)
            nc.sync.dma_start(
                out=wkvt, in_=wkv[l, a].rearrange("(k p) e -> p k e", p=128))
            ubkv_row = row_tile(ubkv[l, a], 2 * E)
            yield

            PHASES.append((f"attn{l}.{a}.alpha", len(nc.inst_map)))
            kv_acc = pat.tile([128, KE, 129], F32, tag="kva", bufs=2)

            xpf = [None] * NC
            if prefetch:
                xpf[0] = load_x_chunk(xkv_dram, 0)
            k2fs = [None] * NC
            v2xs = [None] * NC

            def alpha1(ci):
                if prefetch and ci + 1 < NC:
                    xpf[ci + 1] = load_x_chunk(xkv_dram, ci + 1)
                if xpf[ci] is None:
                    xpf[ci] = load_x_chunk(xkv_dram, ci)
                xt = xpf[ci]
                xpf[ci] = None
                k2f = pa.tile([128, NTT, E], BF16, tag="a4")
                v2x = pa.tile([128, NTT, 4, 129], BF16, tag="a4")
                v.memset(v2x[:, :, :, 128:129], 1.0)
                for tt in range(NTT):
                    xs = xt[:, :, tt * 128:(tt + 1) * 128]
                    pk = ps.tile([128, E], F32, tag="mm")
                    pv = ps.tile([128, E], F32, tag="mm")
                    for k in range(KE):
                        mm(pk, xs[:, k, :], wkvt[:, k, 0:E],
                           start=(k == 0), stop=False)
                        mm(pv, xs[:, k, :], wkvt[:, k, E:2 * E],
                           start=(k == 0), stop=False)
                    mm(pk, ONES_ROW, ubkv_row[:, 0:E], start=False, stop=True)
                    mm(pv, ONES_ROW, ubkv_row[:, E:2 * E], start=False,
                       stop=True)
                    ee = pc.tile([128, E], BF16, tag="a1")
                    rr = pc.tile([128, E], BF16, tag="a1")
                    sc.activation(ee, pk, AF.Exp)
                    sc.activation(rr, pk, AF.Relu)
                    m1 = pc.tile([128, E], BF16, tag="a1")
                    gp.tensor_scalar_min(m1, ee, 1.0)
                    v.tensor_tensor(out=k2f[:, tt, :], in0=m1, in1=rr,
                                    op=ALU.add)
                    for q in range(4):
                        v.tensor_copy(v2x[:, tt, q, 0:128],
                                      pv[:, q * 128:(q + 1) * 128])
                k2fs[ci], v2xs[ci] = k2f, v2x

            def alpha2(ci):
                k2f, v2x = k2fs[ci], v2xs[ci]
                k2fs[ci] = v2xs[ci] = None
                for p in range(4):
                    pkv = ps.tile([128, 129], F32, tag="mm")
                    for tt in range(NTT):
                        mm(pkv, k2f[:, tt, p * 128:(p + 1) * 128],
                           v2x[:, tt, p, :],
                           start=(tt == 0), stop=(tt == NTT - 1))
                    if ci == 0:
                        sc.activation(kv_acc[:, p, :], pkv, AF.Copy)
                    else:
                        v.tensor_tensor(out=kv_acc[:, p, :],
                                        in0=kv_acc[:, p, :], in1=pkv,
                                        op=ALU.add)

            wqt = owt = ubq_col = ob_col = None

            def load_beta_weights():
                nonlocal wqt, owt, ubq_col, ob_col
                wqt = wbig.tile([128, KE, E], BF16, tag="w", bufs=5)
                nc.sync.dma_start(
                    out=wqt,
                    in_=wq[l, a].rearrange("(k p) e -> p k e", p=128))
                owt = wbig.tile([128, KE, E], BF16, tag="w", bufs=5)
                nc.sync.dma_start(
                    out=owt,
                    in_=ow[l, a].rearrange("(k p) e -> p k e", p=128))
                ubq_col = col_tile(ubq[l, a], KE)
                ob_col = col_tile(obf[l, a], KE)

            for ci in range(NC):
                alpha1(ci)
                yield "a1"
                alpha2(ci)
                if ci == NC - 2:
                    load_beta_weights()
                yield "a2"

            bd = pat.tile([128, KE, 128], BF16, tag="bd", bufs=2)
            v.memset(bd, 0.0)
            for p in range(4):
                v.tensor_copy(bd[0:64, p, 0:64], kv_acc[0:64, p, 0:64])
                v.tensor_copy(bd[64:128, p, 64:128],
                              kv_acc[64:128, p, 64:128])
            kmm = pat.tile([128, KE, H], BF16, tag="km")
            for k in range(KE):
                v.tensor_scalar_mul(kmm[:, k, :], hmask_t[:, k, :],
                                    kv_acc[:, k, 128:129])
            yield "bd"

            PHASES.append((f"attn{l}.{a}.beta", len(nc.inst_map)))
            qpf = [None] * NC
            qpf[0] = load_x_chunk(xq_dram, 0)
            qfs = [None] * NC
            xqs = [None] * NC

            def beta1(ci):
                if ci + 1 < NC:
                    qpf[ci + 1] = load_x_chunk(xq_dram, ci + 1)
                xq = qpf[ci]
                qpf[ci] = None
                qf = pa.tile([128, KE, C], BF16, tag="a4")
                for m in range(KE):
                    pq = ps.tile([128, C], F32, tag="mm")
                    for k in range(KE):
                        mm(pq, wqt[:, k, m * 128:(m + 1) * 128], xq[:, k, :],
                           start=(k == 0), stop=(k == KE - 1))
                    ee = pc.tile([128, C], BF16, tag="a1")
                    rr = pc.tile([128, C], BF16, tag="a1")
                    sc.activation(ee, pq, AF.Exp, bias=ubq_col[:, m:m + 1])
                    sc.activation(rr, pq, AF.Relu, bias=ubq_col[:, m:m + 1])
                    m1 = pc.tile([128, C], BF16, tag="a1")
                    gp.tensor_scalar_min(m1, ee, 1.0)
                    v.tensor_tensor(out=qf[:, m, :], in0=m1, in1=rr,
                                    op=ALU.add)
                qfs[ci], xqs[ci] = qf, xq

            for ci in range(NC):
                beta1(ci)
                yield "b1"
                qf, xq = qfs[ci], xqs[ci]
                qfs[ci] = xqs[ci] = None
                pd = psr.tile([8, C], F32, tag="row")
                for k in range(KE):
                    mm(pd, kmm[:, k, :], qf[:, k, :], start=(k == 0),
                       stop=(k == KE - 1))
                rec = pc.tile([8, C], BF16, tag="a1")
                v.reciprocal(out=rec, in_=pd)
                yield "b2a"
                att = pa.tile([128, KE, C], BF16, tag="a4")
                for m in range(KE):
                    pn = ps.tile([128, C], F32, tag="mm")
                    mm(pn, bd[:, m, :], qf[:, m, :], start=True, stop=True)
                    pr = ps.tile([128, C], F32, tag="mm")
                    mm(pr, cmask_t[:, m, :], rec, start=True, stop=True)
                    rb = pc.tile([128, C], BF16, tag="a1")
                    sc.activation(rb, pr, AF.Copy)
                    v.tensor_tensor(out=att[:, m, :], in0=pn, in1=rb,
                                    op=ALU.mult)
                yield "b2b"
                for m in range(KE):
                    pos = ps.tile([128, C], F32, tag="mm")
                    for k in range(KE):
                        mm(pos, owt[:, k, m * 128:(m + 1) * 128],
                           att[:, k, :], start=(k == 0), stop=(k == KE - 1))
                    tail_m(ci, m, pos, xq, ob_col)
                tail_post(ci, xq)
                yield "b3"

        # ---- tails ----
        def make_self_tail(l, s, dst):
            g_col = col_tile(lng[l, 0 if s == "b" else 1], KE, tag="lncol", bufs=16)
            b_col = col_tile(lnb[l, 0 if s == "b" else 1], KE, tag="lncol", bufs=16)
            rt_box = [None]

            sq_box = [None]

            def tail_m(ci, m, pos, xq, ob_col):
                if m == 0:
                    rt_box[0] = pa.tile([128, KE, C], BF16, tag="a4",
                                        name="rt")
                    sq_box[0] = pa.tile([128, KE, C], BF16, tag="a4",
                                        name="rtsq")
                pj = pc.tile([128, C], BF16, tag="a1")
                sc.activation(pj, pos, AF.Identity,
                              bias=ob_col[:, m:m + 1])
                v.tensor_tensor(out=rt_box[0][:, m, :], in0=pj,
                                in1=xq[:, m, :], op=ALU.add)
                v.tensor_tensor(out=sq_box[0][:, m, :],
                                in0=rt_box[0][:, m, :],
                                in1=rt_box[0][:, m, :], op=ALU.mult)

            def tail_post(ci, xq):
                rt = rt_box[0]
                outt = pa.tile([128, KE, C], BF16, tag="a4")
                ln_apply(rt, g_col, b_col, outt, sq=sq_box[0])
                store_chunk(dst, ci, outt)

            return tail_m, tail_post

        def make_cross_tail(l, s, dst):
            gw1t = wsm.tile([128, 2 * KE, E4], BF16, tag="ws")
            nc.sync.dma_start(out=gw1t,
                              in_=gw1[l].rearrange("(k p) g -> p k g", p=128))
            gwd_col = wcol.tile([128, 1], BF16, tag="gwd")
            nc.sync.dma_start(out=gwd_col, in_=gwd[l][:, None])
            gb1_col = col_tile(gb1[l], 1, tag="lncol", bufs=16)
            gb2d_t = pat.tile([1, 1], F32, tag="gb2d")
            nc.sync.dma_start(out=gb2d_t, in_=gb2d[l][None, :])
            g_col = col_tile(lng[l, 2], KE, tag="lncol", bufs=16)
            b_col = col_tile(lnb[l, 2], KE, tag="lncol", bufs=16)
            proj_box = [None]

            def tail_m(ci, m, pos, xq, ob_col):
                if m == 0:
                    proj_box[0] = pa.tile([128, KE, C], BF16, tag="a4",
                                          name="proj")
                sc.activation(proj_box[0][:, m, :], pos, AF.Identity,
                              bias=ob_col[:, m:m + 1])

            def tail_post(ci, xq):
                proj = proj_box[0]
                pg = ps.tile([128, C], F32, tag="mm")
                for k in range(2 * KE):
                    rhs = xq[:, k, :] if k < KE else proj[:, k - KE, :]
                    mm(pg, gw1t[:, k, :], rhs, start=(k == 0),
                       stop=(k == 2 * KE - 1))
                g1 = pc.tile([128, C], BF16, tag="a1")
                sc.activation(g1, pg, AF.Relu, bias=gb1_col[:, 0:1])
                g1t = pc.tile([128, C], BF16, tag="a1")
                v.tensor_scalar_min(g1t, g1, 6.0)
                pg2 = psr.tile([1, C], F32, tag="row")
                mm(pg2, gwd_col, g1t, start=True, stop=True)
                # sigmoid(x + d) = 1/(1 + exp(-x - d)); gb2d_t holds -d
                er = pc.tile([1, C], F32, tag="row", bufs=8)
                sc.activation(er, pg2, AF.Exp, scale=-1.0,
                              bias=gb2d_t[0:1, 0:1])
                u1 = pc.tile([1, C], F32, tag="row", bufs=8)
                v.tensor_scalar_add(u1, er, 1.0)
                bg = pc.tile([1, C], BF16, tag="rowh", bufs=6)
                v.reciprocal(out=bg, in_=u1)
                bgb = pc.tile([128, C], BF16, tag="a1")
                gp.partition_broadcast(bgb, bg)
                mt = pa.tile([128, KE, C], BF16, tag="a4")
                msq = pa.tile([128, KE, C], BF16, tag="a4")
                for m in range(KE):
                    dtmp = pc.tile([128, C], BF16, tag="a1")
                    v.tensor_tensor(out=dtmp, in0=xq[:, m, :],
                                    in1=proj[:, m, :], op=ALU.subtract)
                    v.tensor_tensor(out=dtmp, in0=dtmp, in1=bgb, op=ALU.mult)
                    v.tensor_tensor(out=mt[:, m, :], in0=dtmp,
                                    in1=proj[:, m, :], op=ALU.add)
                    v.tensor_tensor(out=msq[:, m, :], in0=mt[:, m, :],
                                    in1=mt[:, m, :], op=ALU.mult)
                outt = pa.tile([128, KE, C], BF16, tag="a4")
                ln_apply(mt, g_col, b_col, outt, sq=msq)
                store_chunk(dst, ci, outt)

            return tail_m, tail_post

        # ---- FFN ----
        def ffn_gen(l, s, src, dst):
            si = 0 if s == "b" else 1
            w1t = wbig.tile([128, KE, X], BF16, tag="w2x", bufs=4)
            nc.sync.dma_start(
                out=w1t, in_=w1[l, si].rearrange("(k p) x -> p k x", p=128))
            w2t = wbig.tile([128, KX, E], BF16, tag="w2x", bufs=4)
            nc.sync.dma_start(
                out=w2t, in_=w2[l, si].rearrange("(k p) e -> p k e", p=128))
            b1_col = col_tile(b1f[l, si], KX, tag="ffcol", bufs=16)
            b2_col = col_tile(b2f[l, si], KE, tag="ffcol", bufs=16)
            w0_col = col_tile(cwf[l, si, 0], KX, tag="ffcol", bufs=16)
            w1c_col = col_tile(cwf[l, si, 1], KX, tag="ffcol", bufs=16)
            w2_col = col_tile(cwf[l, si, 2], KX, tag="ffcol", bufs=16)
            B_col = col_tile(bnB[l, si], KX, tag="ffcol", bufs=16)
            g_col = col_tile(lng[l, 3 if s == "b" else 4], KE, tag="lncol", bufs=16)
            bb_col = col_tile(lnb[l, 3 if s == "b" else 4], KE, tag="lncol", bufs=16)
            yield
            PHASES.append((f"ffn{l}.{s}", len(nc.inst_map)))

            hts = [None] * NC
            xts = [None] * NC
            hl0 = [None] * NC
            hf2 = [None] * NC

            fpf = [None] * NC

            def compute_h(ci):
                if fpf[ci] is None:
                    fpf[ci] = load_x_chunk(src, ci)
                if ci + 1 < NC:
                    fpf[ci + 1] = load_x_chunk(src, ci + 1)
                xt = fpf[ci]
                xts[ci] = xt
                ht = pb.tile([128, KX, C], BF16, tag="a8")
                for m in range(KX):
                    ph = ps.tile([128, C], F32, tag="mm")
                    for k in range(KE):
                        mm(ph, w1t[:, k, m * 128:(m + 1) * 128], xt[:, k, :],
                           start=(k == 0), stop=(k == KE - 1))
                    hf = pc.tile([128, C], BF16, tag="a1")
                    sc.activation(hf, ph, AF.Relu, bias=b1_col[:, m:m + 1])
                    v.tensor_scalar_min(ht[:, m, :], hf, 6.0)
                hts[ci] = ht
                l0 = phl.tile([128, KX, 1], BF16, tag="hl")
                f2 = phl.tile([128, KX, 1], BF16, tag="hf")
                for m in range(KX):
                    v.tensor_scalar_mul(l0[:, m, :], ht[:, m, C - 1:C],
                                        w0_col[:, m:m + 1])
                    v.tensor_scalar_mul(f2[:, m, :], ht[:, m, 0:1],
                                        w2_col[:, m:m + 1])
                hl0[ci], hf2[ci] = l0, f2

            def conv_elem(ci):
                # taps BN-A-folded. Stage-major in groups of 4 m so each
                # engine runs same-stage ops back-to-back; legs split
                # Act (w0 tap) / DVE (center+adds) / Pool (w2 leg).
                ht = hts[ci]
                h2 = pb.tile([128, KX, C], BF16, tag="a8")
                for m0 in range(0, KX, 4):
                    ms = range(m0, m0 + 4)
                    ta = {}
                    acc = {}
                    for m in ms:
                        ta[m] = pc.tile([128, C], BF16, tag="a1",
                                        name=f"cta{m}")
                        sc.activation(ta[m][:, 0:C - 1], ht[:, m, 0:C - 1],
                                      AF.Copy, scale=w0_col[:, m:m + 1])
                    for m in ms:
                        acc[m] = pc.tile([128, C], BF16, tag="a1",
                                         name=f"cacc{m}")
                        v.tensor_scalar_mul(acc[m], ht[:, m, :],
                                            w1c_col[:, m:m + 1])
                    for m in ms:
                        v.tensor_tensor(out=acc[m][:, 1:C],
                                        in0=acc[m][:, 1:C],
                                        in1=ta[m][:, 0:C - 1], op=ALU.add)
                        if ci > 0:
                            v.tensor_tensor(out=acc[m][:, 0:1],
                                            in0=acc[m][:, 0:1],
                                            in1=hl0[ci - 1][:, m, :],
                                            op=ALU.add)
                    tb = {}
                    for m in ms:
                        tb[m] = pc.tile([128, C], BF16, tag="a1",
                                        name=f"ctb{m}")
                        gp.tensor_scalar_mul(tb[m][:, 1:C], ht[:, m, 1:C],
                                             w2_col[:, m:m + 1])
                    for m in ms:
                        v.tensor_tensor(out=acc[m][:, 0:C - 1],
                                        in0=acc[m][:, 0:C - 1],
                                        in1=tb[m][:, 1:C], op=ALU.add)
                    for m in ms:
                        if ci < NC - 1:
                            v.tensor_tensor(out=acc[m][:, C - 1:C],
                                            in0=acc[m][:, C - 1:C],
                                            in1=hf2[ci + 1][:, m, :],
                                            op=ALU.add)
                        a2 = pc.tile([128, C], BF16, tag="a1")
                        sc.activation(a2, acc[m], AF.Relu,
                                      bias=B_col[:, m:m + 1])
                        v.tensor_scalar_min(h2[:, m, :], a2, 6.0)
                return h2

            def conv_pw(ci, h2):
                rt = pa.tile([128, KE, C], BF16, tag="a4")
                rsq = pa.tile([128, KE, C], BF16, tag="a4")
                for m in range(KE):
                    pw = ps.tile([128, C], F32, tag="mm")
                    for k in range(KX):
                        mm(pw, w2t[:, k, m * 128:(m + 1) * 128], h2[:, k, :],
                           start=(k == 0), stop=(k == KX - 1))
                    pj = pc.tile([128, C], BF16, tag="a1")
                    sc.activation(pj, pw, AF.Identity,
                                  bias=b2_col[:, m:m + 1])
                    v.tensor_tensor(out=rt[:, m, :], in0=pj,
                                    in1=xts[ci][:, m, :], op=ALU.add)
                    v.tensor_tensor(out=rsq[:, m, :], in0=rt[:, m, :],
                                    in1=rt[:, m, :], op=ALU.mult)
                outt = pa.tile([128, KE, C], BF16, tag="a4")
                ln_apply(rt, g_col, bb_col, outt, sq=rsq)
                store_chunk(dst, ci, outt)
                hts[ci] = xts[ci] = None

            compute_h(0)
            yield "h"
            compute_h(1)
            yield "h"
            for ci in range(NC - 1):
                h2 = conv_elem(ci)
                yield "cv"
                conv_pw(ci, h2)
                yield "pw"
                if ci + 2 < NC:
                    compute_h(ci + 2)
                    yield "h"
            h2 = conv_elem(NC - 1)
            yield "cv"
            conv_pw(NC - 1, h2)
            yield "pw"

        # ---- final head (as a generator so it weaves into ffn2) ----
        def final_gen():
            fw1t = wbig.tile([128, 2 * KE, E2], BF16, tag="w", bufs=5)
            nc.sync.dma_start(out=fw1t,
                              in_=fw1.rearrange("(k p) g -> p k g", p=128))
            fw2t = wsm.tile([128, 2, E], BF16, tag="wfin", bufs=4)
            nc.sync.dma_start(out=fw2t,
                              in_=fw2.rearrange("(k p) e -> p k e", p=128))
            rw1t = wsm.tile([128, KE, E4], BF16, tag="wfin", bufs=4)
            nc.sync.dma_start(out=rw1t,
                              in_=rw1.rearrange("(k p) g -> p k g", p=128))
            rw2t = wsm.tile([128, E8], BF16, tag="wfin", bufs=4)
            nc.sync.dma_start(out=rw2t, in_=rw2)
            rw3t = wsm.tile([E8, 16], BF16, tag="wfin", bufs=4)
            nc.sync.dma_start(out=rw3t, in_=rw3p)
            rb3_row = row_tile(rb3p, 16)
            fb2_col = col_tile(fb2f, KE, tag="fcol")
            fb1_col = col_tile(fb1, 2, tag="fcol")
            flng_col = col_tile(flng, KE, tag="fcol")
            flnb_col = col_tile(flnb, KE, tag="fcol")
            rb1_col = col_tile(rb1, 1, tag="fcol")
            rb2_col = wcol.tile([E8, 1], F32, tag="fcol")
            nc.sync.dma_start(out=rb2_col, in_=rb2[:, None])
            out_ap = out_dram.ap()
            bsrc, lsrc = rs["b", (L - 1, 3)], rs["l", (L - 1, 3)]
            yield
            PHASES.append(("final", len(nc.inst_map)))

            def final_stage1(ci):
                xb = load_x_chunk(bsrc, ci)
                xl = load_x_chunk(lsrc, ci)
                f1t = pa.tile([128, 2, C], BF16, tag="a4")
                for m in range(2):
                    pf = ps.tile([128, C], F32, tag="mm")
                    for k in range(2 * KE):
                        rhs = xb[:, k, :] if k < KE else xl[:, k - KE, :]
                        mm(pf, fw1t[:, k, m * 128:(m + 1) * 128], rhs,
                           start=(k == 0), stop=(k == 2 * KE - 1))
                    f1 = pc.tile([128, C], BF16, tag="a1")
                    sc.activation(f1, pf, AF.Relu, bias=fb1_col[:, m:m + 1])
                    v.tensor_scalar_min(f1t[:, m, :], f1, 6.0)
                ft = pa.tile([128, KE, C], BF16, tag="a4")
                fsq = pa.tile([128, KE, C], BF16, tag="a4")
                for m in range(KE):
                    pf2 = ps.tile([128, C], F32, tag="mm")
                    for k in range(2):
                        mm(pf2, fw2t[:, k, m * 128:(m + 1) * 128],
                           f1t[:, k, :], start=(k == 0), stop=(k == 1))
                    sc.activation(ft[:, m, :], pf2, AF.Identity,
                                  bias=fb2_col[:, m:m + 1])
                    v.tensor_tensor(out=fsq[:, m, :], in0=ft[:, m, :],
                                    in1=ft[:, m, :], op=ALU.mult)
                frt = pa.tile([128, KE, C], BF16, tag="a4")
                ln_apply(ft, flng_col, flnb_col, frt, sq=fsq)
                # relu after LN
                frf = frt.rearrange("p k c -> p (k c)")
                v.tensor_scalar_max(frf, frf, 0.0)
                return frt

            def final_stage2(ci, frt):
                p1 = ps.tile([128, C], F32, tag="mm")
                for k in range(KE):
                    mm(p1, rw1t[:, k, :], frt[:, k, :], start=(k == 0),
                       stop=(k == KE - 1))
                h1f = pc.tile([128, C], BF16, tag="a1")
                sc.activation(h1f, p1, AF.Relu, bias=rb1_col[:, 0:1])
                h1t = pc.tile([128, C], BF16, tag="a1")
                v.tensor_scalar_min(h1t, h1f, 6.0)
                p2 = ps.tile([E8, C], F32, tag="mm")
                mm(p2, rw2t, h1t, start=True, stop=True)
                h2f = pc.tile([E8, C], BF16, tag="a1")
                sc.activation(h2f, p2, AF.Relu, bias=rb2_col[:, 0:1])
                h2t = pc.tile([E8, C], BF16, tag="a1")
                v.tensor_scalar_min(h2t, h2f, 6.0)
                ot = pc.tile([128, NTT, c.OUT], F32, tag="a1")
                for tt in range(NTT):
                    p3 = ps.tile([128, 16], F32, tag="mm")
                    mm(p3, h2t[:, tt * 128:(tt + 1) * 128], rw3t,
                       start=True, stop=False)
                    mm(p3, ONES_ROW, rb3_row, start=False, stop=True)
                    sc.activation(ot[:, tt, :], p3[:, 0:c.OUT], AF.Copy)
                nc.sync.dma_start(
                    out=out_ap[ci * C:(ci + 1) * C, :].rearrange(
                        "(tt p) o -> p tt o", p=128),
                    in_=ot)

            for ci in range(NC):
                frt = final_stage1(ci)
                yield "s1"
                final_stage2(ci, frt)
                yield "s2"

        # ---- layers (chunk-interleaved across independent streams,
        # next-layer self alphas / final head woven into the FFN) ----
        def adv(g, n=1):
            for _ in range(n):
                next(g, None)

        def make_selfs(l, prefetch):
            bsrc = rs["b", 0] if l == 0 else rs["b", (l - 1, 3)]
            lsrc = rs["l", 0] if l == 0 else rs["l", (l - 1, 3)]
            g0 = attn_gen(l, 0, bsrc, bsrc,
                          *make_self_tail(l, "b", rs["b", (l, 1)]),
                          prefetch=prefetch)
            g1 = attn_gen(l, 1, lsrc, lsrc,
                          *make_self_tail(l, "l", rs["l", (l, 1)]),
                          prefetch=prefetch)
            return g0, g1

        def weave(gb, gl, ext):
            """Dense FFN phase first (keeps PE matmul runs unbroken for
            the pstate ramp), then the ext generators (next-layer self
            alphas or final head) immediately after."""
            for _ in range(11):
                adv(gb); adv(gl)      # through cv3
            for g in ext:
                adv(g)                # ext weight loads (DMA only) overlap
            adv(gb); adv(gl)          # pw3
            for _ in range(9):
                for g in ext:
                    adv(g)

        g0, g1 = make_selfs(0, prefetch=True)
        adv(g0); adv(g1)              # wkv loads
        adv(g0); adv(g1)              # a1(0): x chunk loads + projections
        hmask_t, cmask_t = _late_consts()
        for _ in range(2 * NC - 1):   # rest of layer-0 self alphas
            adv(g0); adv(g1)
        adv(g0); adv(g1)              # bd + wq/ow loads

        for l in range(L):
            g2 = attn_gen(l, 2, rs["b", (l, 1)], rs["l", (l, 1)],
                          *make_cross_tail(l, "b", rs["b", (l, 2)]))
            g3 = attn_gen(l, 3, rs["l", (l, 1)], rs["b", (l, 1)],
                          *make_cross_tail(l, "l", rs["l", (l, 2)]))
            gb = ffn_gen(l, "b", rs["b", (l, 2)], rs["b", (l, 3)])
            gl = ffn_gen(l, "l", rs["l", (l, 2)], rs["l", (l, 3)])
            for _ in range(2):        # first self-beta steps (queue the
                adv(g0); adv(g1)      # chunk loads ahead of cross weights)
            adv(g2); adv(g3)          # cross wkv loads
            for _ in range(4 * NC - 2):
                adv(g0); adv(g1)      # rest of self betas
            for _ in range(2 * NC):   # cross alphas
                adv(g2); adv(g3)
            adv(g2); adv(g3)          # cross bd + wq/ow
            for _ in range(2):        # first cross-beta steps
                adv(g2); adv(g3)
            adv(gb); adv(gl)          # ffn weights (behind first q loads)
            for _ in range(4 * NC - 2):
                adv(g2); adv(g3)      # rest of cross betas
            if l + 1 < L:
                ng0, ng1 = make_selfs(l + 1, prefetch=True)
                weave(gb, gl, [ng0, ng1])
                g0, g1 = ng0, ng1
            else:
                weave(gb, gl, [final_gen()])

    return din, out_dram


# ======================================================================
# kernel() entry point: full inputs in, full outputs out (8-core SPMD).
# ======================================================================
import concourse.bacc as _bacc
from concourse.bass_utils import run_bass_kernel_spmd as _run_spmd

_N_CORES = 8
_CACHE = {}


def _steer_act_tables(nc):
    """Steer the act-table selection pass toward the single set that
    contains every activation function this kernel uses (ln, exp, relu,
    copy, identity, square), so one table load serves the whole program.
    The greedy pass otherwise alternates between the exp-only and ln-only
    sets, emitting ~150 LoadActFuncSet instructions (~1.3us each) that
    serialize the Act queue. Set ids stay aligned with act_info.json, so
    the emitted id remains valid for walrus."""
    from concourse.hw_specs import get_activation_tables
    AF = mybir.ActivationFunctionType
    need = {AF.Ln, AF.Exp, AF.Relu, AF.Copy, AF.Identity, AF.Square}
    try:
        tabs = get_activation_tables(nc.m.arch)
    except Exception:
        return
    best = None
    for name, funcs in tabs.items():
        if need <= funcs:
            best = name
            break
    if best is None:
        return
    for name in tabs:
        if name != best:
            tabs[name] = set()


def _get_nc():
    if "nc" not in _CACHE:
        nc = _bacc.Bacc("TRN2", target_bir_lowering=False, debug=False)
        _steer_act_tables(nc)
        build(nc, Cfg())
        nc.finalize()
        _CACHE["nc"] = nc
    return _CACHE["nc"]


def _bf16(x):
    import ml_dtypes
    return np.asarray(x, dtype=np.float32).astype(ml_dtypes.bfloat16)


def host_prep(inputs):
    """Host-side weight preprocessing: compose QKV, fold BN, convert bf16."""
    c = Cfg()
    E, X, H, L = c.E, c.X, c.H, c.L
    E4, E2, E8 = E // 4, E // 2, E // 8
    f = {k: np.asarray(v, dtype=np.float32) for k, v in inputs.items()}
    dw, uw, ub = f["dw"], f["uw"], f["ub"]
    # composed q and k|v projection weights
    wq = np.matmul(dw[:, :, 0], uw[:, :, 0])          # (L,4,E,E)
    wk = np.matmul(dw[:, :, 1], uw[:, :, 1])
    wv = np.matmul(dw[:, :, 2], uw[:, :, 2])
    wkv = np.concatenate([wk, wv], axis=-1)           # (L,4,E,2E)
    ubq = ub[:, :, 0]                                 # (L,4,E)
    ubkv = np.concatenate([ub[:, :, 1], ub[:, :, 2]], axis=-1)
    rsq = np.float32(1.0 / np.sqrt(1.0 + BN_EPS))
    A = f["bng"] * rsq                                # (L,2,X)
    B = f["cb"] * A + f["bnb"]
    # conv taps tap-major with BN scale A folded in
    cwf = f["cw"].transpose(0, 1, 3, 2) * A[:, :, None, :]  # (L,2,3,X)
    gwd = f["gw2"][:, :, 0] - f["gw2"][:, :, 1]       # (L,E4)
    # negated: kernel computes sigmoid via exp(-x + bias) with bias = -d
    gb2d = -(f["gb2"][:, 0] - f["gb2"][:, 1])[:, None]  # (L,1)
    rw3p = np.zeros((E8, 16), np.float32)
    rw3p[:, :c.OUT] = f["rw3"]
    rb3p = np.zeros((16,), np.float32)
    rb3p[:c.OUT] = f["rb3"]
    dh = E // H
    ident = np.eye(128, dtype=np.float32)
    ones = np.ones((128, 128), dtype=np.float32)
    hmask = np.zeros((E, H), dtype=np.float32)
    for ff in range(E):
        hmask[ff, ff // dh] = 1.0
    cmask = hmask.T.copy()

    b16 = dict(wq=wq, wkv=wkv, ubkv=ubkv, ow=f["ow"],
               w1=f["w1"], w2=f["w2"],
               gw1=f["gw1"], gwd=gwd, fw1=f["fw1"], fw2=f["fw2"],
               rw1=f["rw1"], rw2=f["rw2"], rw3p=rw3p,
               rb3p=rb3p, ident=ident, ones128=ones, hmask=hmask,
               cmask=cmask)
    f32 = dict(ubq=ubq, ob=f["ob"], b1=f["b1"], b2=f["b2"], fb2=f["fb2"],
               cwf=cwf, bnB=B,
               lng=f["lng"], lnb=f["lnb"],
               gb1=f["gb1"], gb2d=gb2d, fb1=f["fb1"], flng=f["flng"],
               flnb=f["flnb"], rb1=f["rb1"], rb2=f["rb2"])
    shared = {k: _bf16(v) for k, v in b16.items()}
    shared.update({k: np.ascontiguousarray(v, dtype=np.float32)
                   for k, v in f32.items()})
    return shared, f["body_feats"], f["limb_feats"]


def kernel(**inputs):
    nc = _get_nc()
    shared, body, limb = host_prep(inputs)
    in_maps = []
    for i in range(_N_CORES):
        m = dict(shared)
        m["body_feats"] = np.ascontiguousarray(_bf16(body[i]).T)
        m["limb_feats"] = np.ascontiguousarray(_bf16(limb[i]).T)
        in_maps.append(m)
    res = run_kernel_spmd_cached(nc, in_maps)
    out = np.stack([res[i]["out"] for i in range(_N_CORES)], axis=0)
    return out.astype(np.float32)


def run_kernel_spmd_cached(nc, in_maps, **kw):
    r = _run_spmd(nc, in_maps, list(range(_N_CORES)), **kw)
    _CACHE["last_result"] = r
    return r.results



# revision 88
# speedup vs baseline: 1.0061x; 1.0061x over previous
"""Dual-stream linear-attention transformer — bf16 redesign (per-core).

Layout convention (same as baseline):
  - "layout 1" activation: [E, N] feature-major; SBUF tiles [128, KE, C]
    (feature f = 128*k + p -> partition p, k-th slice; tokens on free dim).
  - alpha k/v are produced token-major per 128-token tile [128, E].
  - Residual streams live in internal DRAM as [E, N] bf16.

Key design points:
  - All matmul operands + SBUF activations bf16; biases folded into
    matmuls (rank-1 ones_row accumulates) or Act per-partition bias.
  - QKV down+up projections composed into single E x E / E x 2E weights
    on the host.
  - Every Act function stays inside ONE activation-table set
    (natural_log_exp_and_others): LayerNorm rsqrt = exp(-0.5*ln(v+eps)),
    gating sigmoid = 1/(1+exp(-x)) via DVE reciprocal, and
    _steer_act_tables biases the table-selection pass so exactly one
    LoadActFuncSet is emitted (the greedy pass otherwise thrashes
    ~84-150 reloads x 1.3us between the exp- and ln-only sets).
  - elu+1 split Act/Pool/DVE: Act Exp + Act Relu, Pool min(.,1), DVE add.
  - FFN dwconv taps are BN-A-folded on the host; per-m conv legs are
    stage-major and split Act (w0 tap via per-partition-scale Copy),
    DVE (center tap, adds, halos, relu6 min) and Pool (w2 tap mul).
  - LN tails: residual add via Act Identity(bias)+DVE TT, squares
    emitted per slice so the stats matmuls start early.
  - Scheduling is deliberately phase-DENSE (no fine interleaving): the
    TimelineSim PE pstate model runs matmuls at 2.4GHz only after ~3us
    of continuous PE activity (788ns cold / 427ns warm / 213ns hot per
    512-col bf16 matmul), so long unbroken matmul bursts beat any
    work-spreading weave. Cross-phase overlap via DRAM store->load
    roundtrips always lost: an inserted stage that is not instantly
    ready head-of-line blocks the in-order PE queue.
  - Beta-side attention weights (wq/ow) load during late alpha chunks;
    hardware limits honored: Pool has no scalar_tensor_tensor, DVE
    tensor ops may read at most one PSUM operand.
"""

from dataclasses import dataclass
from contextlib import ExitStack

import numpy as np

import concourse.bass as bass
import concourse.mybir as mybir
import concourse.tile as tile

F32 = mybir.dt.float32
BF16 = mybir.dt.bfloat16
AF = mybir.ActivationFunctionType
ALU = mybir.AluOpType

LN_EPS = 1e-5
BN_EPS = 1e-5


@dataclass
class Cfg:
    N: int = 2048
    E: int = 512
    R: int = 256
    X: int = 1024
    H: int = 8
    L: int = 3
    OUT: int = 15
    C: int = 512

    @property
    def KE(self):
        return self.E // 128

    @property
    def KX(self):
        return self.X // 128

    @property
    def NC(self):
        return self.N // self.C

    @property
    def NTT(self):
        return self.C // 128


PHASES = []


def build(nc, cfg):
    c = cfg
    E, X, H, N, C, L = c.E, c.X, c.H, c.N, c.C, c.L
    KE, KX, NC, NTT = c.KE, c.KX, c.NC, c.NTT
    E4, E2, E8 = E // 4, E // 2, E // 8

    din = {}

    def inp(name, shape, dt=BF16):
        din[name] = nc.dram_tensor(name, list(shape), dt, kind="ExternalInput")
        return din[name].ap()

    # activations (host converts to bf16 and pre-transposes to [E, N])
    body_feats = inp("body_feats", (E, N))
    limb_feats = inp("limb_feats", (E, N))
    # attention weights (host-composed)
    wq = inp("wq", (L, 4, E, E))
    wkv = inp("wkv", (L, 4, E, 2 * E))
    ubq = inp("ubq", (L, 4, E), F32)
    ubkv = inp("ubkv", (L, 4, 2 * E))
    ow = inp("ow", (L, 4, E, E))
    obf = inp("ob", (L, 4, E), F32)
    # FFN
    w1 = inp("w1", (L, 2, E, X))
    b1f = inp("b1", (L, 2, X), F32)
    cwf = inp("cwf", (L, 2, 3, X), F32)     # conv taps, tap-major, BN-A folded
    bnB = inp("bnB", (L, 2, X), F32)        # cb*A + bnb
    w2 = inp("w2", (L, 2, X, E))
    b2f = inp("b2", (L, 2, E), F32)
    lng = inp("lng", (L, 5, E), F32)
    lnb = inp("lnb", (L, 5, E), F32)
    # gating
    gw1 = inp("gw1", (L, 2 * E, E4))
    gb1 = inp("gb1", (L, E4), F32)
    gwd = inp("gwd", (L, E4))               # gw2[:,0]-gw2[:,1]
    gb2d = inp("gb2d", (L, 1), F32)         # gb2[0]-gb2[1]
    # final head
    fw1 = inp("fw1", (2 * E, E2))
    fb1 = inp("fb1", (E2,), F32)
    fw2 = inp("fw2", (E2, E))
    fb2f = inp("fb2", (E,), F32)
    flng = inp("flng", (E,), F32)
    flnb = inp("flnb", (E,), F32)
    rw1 = inp("rw1", (E, E4))
    rb1 = inp("rb1", (E4,), F32)
    rw2 = inp("rw2", (E4, E8))
    rb2 = inp("rb2", (E8,), F32)
    rw3p = inp("rw3p", (E8, 16))            # zero-padded to 16
    rb3p = inp("rb3p", (16,))               # zero-padded
    ident_in = inp("ident", (128, 128))
    ones_in = inp("ones128", (128, 128))
    hmask_in = inp("hmask", (E, H))
    cmask_in = inp("cmask", (H, E))

    out_dram = nc.dram_tensor("out", [N, c.OUT], F32, kind="ExternalOutput")

    def idram(name):
        return nc.dram_tensor(name, [E, N], BF16).ap().rearrange(
            "(k p) n -> p k n", p=128)

    rs = {}
    for s in ("b", "l"):
        for l in range(L):
            for st in (1, 2, 3):
                rs[s, (l, st)] = idram(f"r{s}_{l}_{st}")
    rs["b", 0] = body_feats.rearrange("(k p) n -> p k n", p=128)
    rs["l", 0] = limb_feats.rearrange("(k p) n -> p k n", p=128)

    lowp = nc.allow_low_precision("bf16 activations within rel-err budget")

    with tile.TileContext(nc) as tc, ExitStack() as ctx, lowp:
        p_ = ctx.enter_context
        cst = p_(tc.tile_pool(name="cst", bufs=1))
        wbig = p_(tc.tile_pool(name="wbig", bufs=3))
        wsm = p_(tc.tile_pool(name="wsm", bufs=2))
        wcol = p_(tc.tile_pool(name="wcol", bufs=10))
        wrow = p_(tc.tile_pool(name="wrow", bufs=4))
        pa = p_(tc.tile_pool(name="pa", bufs=10))      # 4KB bf16 chunk tiles
        pb = p_(tc.tile_pool(name="pb", bufs=5))      # 8KB ht tiles
        pc = p_(tc.tile_pool(name="pc", bufs=12))      # 1KB bf16 / rows
        pat = p_(tc.tile_pool(name="pat", bufs=3))    # per-attn persistents
        phl = p_(tc.tile_pool(name="phl", bufs=6))    # conv halos
        ps = p_(tc.tile_pool(name="ps", bufs=6, space="PSUM"))
        psr = p_(tc.tile_pool(name="psr", bufs=2, space="PSUM"))

        v, sc, gp = nc.vector, nc.scalar, nc.gpsimd

        def mm(out, lhsT, rhs, start, stop):
            nc.tensor.matmul(out, lhsT, rhs, start=start, stop=stop)

        # ---- constants (ones first: the alpha bias matmuls need it;
        # hmask/cmask/ident aren't read until the bd/beta stages, so their
        # loads queue behind the first weight + activation chunks) ----
        ones_t = cst.tile([128, 128], BF16, tag="ones")
        nc.sync.dma_start(out=ones_t, in_=ones_in)
        def _late_consts():
            ident_t = cst.tile([128, 128], BF16, tag="ident")
            nc.sync.dma_start(out=ident_t, in_=ident_in)
            hmask_t = cst.tile([128, KE, H], BF16, tag="hmask")
            nc.sync.dma_start(out=hmask_t,
                              in_=hmask_in.rearrange("(k p) h -> p k h",
                                                     p=128))
            cmask_t = cst.tile([H, KE, 128], BF16, tag="cmask")
            nc.sync.dma_start(out=cmask_t,
                              in_=cmask_in.rearrange("h (k p) -> h k p",
                                                     p=128))
            return hmask_t, cmask_t
        ONES_COL = ones_t[:, 0:1]
        ONES_ROW = ones_t[0:1, :]
        onesc_t = cst.tile([1, C], BF16, tag="onesc")
        v.memset(onesc_t, 1.0)
        ONES_C = onesc_t[0:1, :]
        eps_ln = cst.tile([1, 1], F32, tag="epsl")
        v.memset(eps_ln, LN_EPS)

        def col_tile(src_ap, m, tag="col", bufs=None):
            t = wcol.tile([128, m], F32, tag=tag,
                          bufs=(12 if tag == "col" else bufs))
            nc.sync.dma_start(out=t, in_=src_ap.rearrange("(m p) -> p m", p=128))
            return t

        def row_tile(src_ap, n, tag="row", pool=None):
            t = (pool or wrow).tile([1, n], BF16, tag=tag)
            nc.sync.dma_start(out=t, in_=src_ap[None, :])
            return t

        def ln_apply(xs, g_col, b_col, outt, sq=None):
            """LayerNorm over features (layout 1). xs: [128, KE, C] bf16 tile.
            outt: [128, KE, C] bf16 out. rsqrt via exp(-0.5*ln(v+eps)) keeps
            every Act func in one table set (no LoadActFuncSet thrash).
            Callers that produce xs slice-by-slice pass a precomputed sq so
            the stats matmuls start after the first slice, not the last."""
            if sq is None:
                sq = pa.tile([128, KE, C], BF16, tag="a4")
                xf = xs.rearrange("p k c -> p (k c)")
                v.tensor_tensor(out=sq.rearrange("p k c -> p (k c)"),
                                in0=xf, in1=xf, op=ALU.mult)
            ps_s = psr.tile([1, C], F32, tag="row")
            ps_ss = psr.tile([1, C], F32, tag="row")
            for m in range(KE):
                mm(ps_s, ONES_COL, xs[:, m, :], start=(m == 0),
                   stop=(m == KE - 1))
                mm(ps_ss, ONES_COL, sq[:, m, :], start=(m == 0),
                   stop=(m == KE - 1))
            mrow = pc.tile([1, C], F32, tag="row", bufs=8)
            v.tensor_scalar_mul(mrow, ps_s, 1.0 / E)
            m2 = pc.tile([1, C], F32, tag="row", bufs=8)
            v.tensor_tensor(out=m2, in0=mrow, in1=mrow, op=ALU.mult)
            var = pc.tile([1, C], F32, tag="row", bufs=8)
            v.scalar_tensor_tensor(out=var, in0=ps_ss, scalar=1.0 / E,
                                   in1=m2, op0=ALU.mult, op1=ALU.subtract)
            lnv = pc.tile([1, C], F32, tag="row", bufs=8)
            sc.activation(lnv, var, AF.Ln, bias=eps_ln[0:1, 0:1])
            srow = pc.tile([1, C], BF16, tag="rowh", bufs=6)
            sc.activation(srow, lnv, AF.Exp, scale=-0.5)
            trow = pc.tile([1, C], BF16, tag="rowh", bufs=6)
            v.scalar_tensor_tensor(out=trow, in0=ps_s, scalar=1.0 / E,
                                   in1=srow, op0=ALU.mult, op1=ALU.mult)
            sb_s = pc.tile([128, C], BF16, tag="a1")
            gp.partition_broadcast(sb_s, srow)
            sb_t = pc.tile([128, C], BF16, tag="a1")
            gp.partition_broadcast(sb_t, trow)
            for m in range(KE):
                u = pc.tile([128, C], BF16, tag="a1")
                v.tensor_tensor(out=u, in0=xs[:, m, :], in1=sb_s, op=ALU.mult)
                v.tensor_tensor(out=u, in0=u, in1=sb_t, op=ALU.subtract)
                v.tensor_scalar(out=outt[:, m, :], in0=u,
                                scalar1=g_col[:, m:m + 1],
                                scalar2=b_col[:, m:m + 1],
                                op0=ALU.mult, op1=ALU.add)

        def load_x_chunk(dram_l1, ci, tag="a4"):
            xt = pa.tile([128, KE, C], BF16, tag=tag)
            nc.sync.dma_start(out=xt, in_=dram_l1[:, :, ci * C:(ci + 1) * C])
            return xt

        def store_chunk(dram_l1, ci, t):
            gp.dma_start(out=dram_l1[:, :, ci * C:(ci + 1) * C], in_=t)


        # ---- linear attention ----
        def attn_gen(l, a, xq_dram, xkv_dram, tail_m, tail_post,
                     prefetch=True):
            """Generator stages: wkv loads | a1/a2 per chunk | bd (+ wq/ow
            loads) | b1/b2a/b2b/b3 per chunk. With prefetch=False the alpha
            x-chunk loads are issued only at their own stage, so a load
            never waits at the DMA queue head for a producer store that
            has not happened yet (used when interleaved with the producing
            FFN)."""
            wkvt = wbig.tile([128, KE, 2 * E], BF16, tag="w2x", bufs=4)
            nc.sync.dma_start(
                out=wkvt, in_=wkv[l, a].rearrange("(k p) e -> p k e", p=128))
            ubkv_row = row_tile(ubkv[l, a], 2 * E)
            yield

            PHASES.append((f"attn{l}.{a}.alpha", len(nc.inst_map)))
            kv_acc = pat.tile([128, KE, 129], F32, tag="kva", bufs=2)

            xpf = [None] * NC
            if prefetch:
                xpf[0] = load_x_chunk(xkv_dram, 0)
            k2fs = [None] * NC
            v2xs = [None] * NC

            def alpha1(ci):
                if prefetch and ci + 1 < NC:
                    xpf[ci + 1] = load_x_chunk(xkv_dram, ci + 1)
                if xpf[ci] is None:
                    xpf[ci] = load_x_chunk(xkv_dram, ci)
                xt = xpf[ci]
                xpf[ci] = None
                k2f = pa.tile([128, NTT, E], BF16, tag="a4")
                v2x = pa.tile([128, NTT, 4, 129], BF16, tag="a4")
                v.memset(v2x[:, :, :, 128:129], 1.0)
                for tt in range(NTT):
                    xs = xt[:, :, tt * 128:(tt + 1) * 128]
                    pk = ps.tile([128, E], F32, tag="mm")
                    pv = ps.tile([128, E], F32, tag="mm")
                    for k in range(KE):
                        mm(pk, xs[:, k, :], wkvt[:, k, 0:E],
                           start=(k == 0), stop=False)
                        mm(pv, xs[:, k, :], wkvt[:, k, E:2 * E],
                           start=(k == 0), stop=False)
                    mm(pk, ONES_ROW, ubkv_row[:, 0:E], start=False, stop=True)
                    mm(pv, ONES_ROW, ubkv_row[:, E:2 * E], start=False,
                       stop=True)
                    ee = pc.tile([128, E], BF16, tag="a1")
                    rr = pc.tile([128, E], BF16, tag="a1")
                    sc.activation(ee, pk, AF.Exp)
                    sc.activation(rr, pk, AF.Relu)
                    m1 = pc.tile([128, E], BF16, tag="a1")
                    gp.tensor_scalar_min(m1, ee, 1.0)
                    v.tensor_tensor(out=k2f[:, tt, :], in0=m1, in1=rr,
                                    op=ALU.add)
                    for q in range(4):
                        v.tensor_copy(v2x[:, tt, q, 0:128],
                                      pv[:, q * 128:(q + 1) * 128])
                k2fs[ci], v2xs[ci] = k2f, v2x

            def alpha2(ci):
                k2f, v2x = k2fs[ci], v2xs[ci]
                k2fs[ci] = v2xs[ci] = None
                for p in range(4):
                    pkv = ps.tile([128, 129], F32, tag="mm")
                    for tt in range(NTT):
                        mm(pkv, k2f[:, tt, p * 128:(p + 1) * 128],
                           v2x[:, tt, p, :],
                           start=(tt == 0), stop=(tt == NTT - 1))
                    if ci == 0:
                        sc.activation(kv_acc[:, p, :], pkv, AF.Copy)
                    else:
                        v.tensor_tensor(out=kv_acc[:, p, :],
                                        in0=kv_acc[:, p, :], in1=pkv,
                                        op=ALU.add)

            wqt = owt = ubq_col = ob_col = None

            def load_beta_weights():
                nonlocal wqt, owt, ubq_col, ob_col
                wqt = wbig.tile([128, KE, E], BF16, tag="w", bufs=5)
                nc.sync.dma_start(
                    out=wqt,
                    in_=wq[l, a].rearrange("(k p) e -> p k e", p=128))
                owt = wbig.tile([128, KE, E], BF16, tag="w", bufs=5)
                nc.sync.dma_start(
                    out=owt,
                    in_=ow[l, a].rearrange("(k p) e -> p k e", p=128))
                ubq_col = col_tile(ubq[l, a], KE)
                ob_col = col_tile(obf[l, a], KE)

            for ci in range(NC):
                alpha1(ci)
                yield "a1"
                alpha2(ci)
                if ci == NC - 2:
                    load_beta_weights()
                yield "a2"

            bd = pat.tile([128, KE, 128], BF16, tag="bd", bufs=2)
            v.memset(bd, 0.0)
            for p in range(4):
                v.tensor_copy(bd[0:64, p, 0:64], kv_acc[0:64, p, 0:64])
                v.tensor_copy(bd[64:128, p, 64:128],
                              kv_acc[64:128, p, 64:128])
            kmm = pat.tile([128, KE, H], BF16, tag="km")
            for k in range(KE):
                v.tensor_scalar_mul(kmm[:, k, :], hmask_t[:, k, :],
                                    kv_acc[:, k, 128:129])
            yield "bd"

            PHASES.append((f"attn{l}.{a}.beta", len(nc.inst_map)))
            qpf = [None] * NC
            qpf[0] = load_x_chunk(xq_dram, 0)
            qfs = [None] * NC
            xqs = [None] * NC

            def beta1(ci):
                if ci + 1 < NC:
                    qpf[ci + 1] = load_x_chunk(xq_dram, ci + 1)
                xq = qpf[ci]
                qpf[ci] = None
                qf = pa.tile([128, KE, C], BF16, tag="a4")
                for m in range(KE):
                    pq = ps.tile([128, C], F32, tag="mm")
                    for k in range(KE):
                        mm(pq, wqt[:, k, m * 128:(m + 1) * 128], xq[:, k, :],
                           start=(k == 0), stop=(k == KE - 1))
                    ee = pc.tile([128, C], BF16, tag="a1")
                    rr = pc.tile([128, C], BF16, tag="a1")
                    sc.activation(ee, pq, AF.Exp, bias=ubq_col[:, m:m + 1])
                    sc.activation(rr, pq, AF.Relu, bias=ubq_col[:, m:m + 1])
                    m1 = pc.tile([128, C], BF16, tag="a1")
                    gp.tensor_scalar_min(m1, ee, 1.0)
                    v.tensor_tensor(out=qf[:, m, :], in0=m1, in1=rr,
                                    op=ALU.add)
                qfs[ci], xqs[ci] = qf, xq

            for ci in range(NC):
                beta1(ci)
                yield "b1"
                qf, xq = qfs[ci], xqs[ci]
                qfs[ci] = xqs[ci] = None
                pd = psr.tile([8, C], F32, tag="row")
                for k in range(KE):
                    mm(pd, kmm[:, k, :], qf[:, k, :], start=(k == 0),
                       stop=(k == KE - 1))
                rec = pc.tile([8, C], BF16, tag="a1")
                v.reciprocal(out=rec, in_=pd)
                yield "b2a"
                att = pa.tile([128, KE, C], BF16, tag="a4")
                for m in range(KE):
                    pn = ps.tile([128, C], F32, tag="mm")
                    mm(pn, bd[:, m, :], qf[:, m, :], start=True, stop=True)
                    pr = ps.tile([128, C], F32, tag="mm")
                    mm(pr, cmask_t[:, m, :], rec, start=True, stop=True)
                    rb = pc.tile([128, C], BF16, tag="a1")
                    sc.activation(rb, pr, AF.Copy)
                    v.tensor_tensor(out=att[:, m, :], in0=pn, in1=rb,
                                    op=ALU.mult)
                yield "b2b"
                for m in range(KE):
                    pos = ps.tile([128, C], F32, tag="mm")
                    for k in range(KE):
                        mm(pos, owt[:, k, m * 128:(m + 1) * 128],
                           att[:, k, :], start=(k == 0), stop=(k == KE - 1))
                    tail_m(ci, m, pos, xq, ob_col)
                tail_post(ci, xq)
                yield "b3"

        # ---- tails ----
        def make_self_tail(l, s, dst):
            g_col = col_tile(lng[l, 0 if s == "b" else 1], KE, tag="lncol", bufs=16)
            b_col = col_tile(lnb[l, 0 if s == "b" else 1], KE, tag="lncol", bufs=16)
            rt_box = [None]

            sq_box = [None]

            def tail_m(ci, m, pos, xq, ob_col):
                if m == 0:
                    rt_box[0] = pa.tile([128, KE, C], BF16, tag="a4",
                                        name="rt")
                    sq_box[0] = pa.tile([128, KE, C], BF16, tag="a4",
                                        name="rtsq")
                pj = pc.tile([128, C], BF16, tag="a1")
                sc.activation(pj, pos, AF.Identity,
                              bias=ob_col[:, m:m + 1])
                v.tensor_tensor(out=rt_box[0][:, m, :], in0=pj,
                                in1=xq[:, m, :], op=ALU.add)
                v.tensor_tensor(out=sq_box[0][:, m, :],
                                in0=rt_box[0][:, m, :],
                                in1=rt_box[0][:, m, :], op=ALU.mult)

            def tail_post(ci, xq):
                rt = rt_box[0]
                outt = pa.tile([128, KE, C], BF16, tag="a4")
                ln_apply(rt, g_col, b_col, outt, sq=sq_box[0])
                store_chunk(dst, ci, outt)

            return tail_m, tail_post

        def make_cross_tail(l, s, dst):
            gw1t = wsm.tile([128, 2 * KE, E4], BF16, tag="ws")
            nc.sync.dma_start(out=gw1t,
                              in_=gw1[l].rearrange("(k p) g -> p k g", p=128))
            gwd_col = wcol.tile([128, 1], BF16, tag="gwd")
            nc.sync.dma_start(out=gwd_col, in_=gwd[l][:, None])
            gb1_col = col_tile(gb1[l], 1, tag="lncol", bufs=16)
            gb2d_t = pat.tile([1, 1], F32, tag="gb2d")
            nc.sync.dma_start(out=gb2d_t, in_=gb2d[l][None, :])
            g_col = col_tile(lng[l, 2], KE, tag="lncol", bufs=16)
            b_col = col_tile(lnb[l, 2], KE, tag="lncol", bufs=16)
            proj_box = [None]

            def tail_m(ci, m, pos, xq, ob_col):
                if m == 0:
                    proj_box[0] = pa.tile([128, KE, C], BF16, tag="a4",
                                          name="proj")
                sc.activation(proj_box[0][:, m, :], pos, AF.Identity,
                              bias=ob_col[:, m:m + 1])

            def tail_post(ci, xq):
                proj = proj_box[0]
                pg = ps.tile([128, C], F32, tag="mm")
                for k in range(2 * KE):
                    rhs = xq[:, k, :] if k < KE else proj[:, k - KE, :]
                    mm(pg, gw1t[:, k, :], rhs, start=(k == 0),
                       stop=(k == 2 * KE - 1))
                g1 = pc.tile([128, C], BF16, tag="a1")
                sc.activation(g1, pg, AF.Relu, bias=gb1_col[:, 0:1])
                g1t = pc.tile([128, C], BF16, tag="a1")
                v.tensor_scalar_min(g1t, g1, 6.0)
                pg2 = psr.tile([1, C], F32, tag="row")
                mm(pg2, gwd_col, g1t, start=True, stop=True)
                # sigmoid(x + d) = 1/(1 + exp(-x - d)); gb2d_t holds -d
                er = pc.tile([1, C], F32, tag="row", bufs=8)
                sc.activation(er, pg2, AF.Exp, scale=-1.0,
                              bias=gb2d_t[0:1, 0:1])
                u1 = pc.tile([1, C], F32, tag="row", bufs=8)
                v.tensor_scalar_add(u1, er, 1.0)
                bg = pc.tile([1, C], BF16, tag="rowh", bufs=6)
                v.reciprocal(out=bg, in_=u1)
                bgb = pc.tile([128, C], BF16, tag="a1")
                gp.partition_broadcast(bgb, bg)
                mt = pa.tile([128, KE, C], BF16, tag="a4")
                msq = pa.tile([128, KE, C], BF16, tag="a4")
                for m in range(KE):
                    dtmp = pc.tile([128, C], BF16, tag="a1")
                    v.tensor_tensor(out=dtmp, in0=xq[:, m, :],
                                    in1=proj[:, m, :], op=ALU.subtract)
                    v.tensor_tensor(out=dtmp, in0=dtmp, in1=bgb, op=ALU.mult)
                    v.tensor_tensor(out=mt[:, m, :], in0=dtmp,
                                    in1=proj[:, m, :], op=ALU.add)
                    v.tensor_tensor(out=msq[:, m, :], in0=mt[:, m, :],
                                    in1=mt[:, m, :], op=ALU.mult)
                outt = pa.tile([128, KE, C], BF16, tag="a4")
                ln_apply(mt, g_col, b_col, outt, sq=msq)
                store_chunk(dst, ci, outt)

            return tail_m, tail_post

        # ---- FFN ----
        def ffn_gen(l, s, src, dst):
            si = 0 if s == "b" else 1
            w1t = wbig.tile([128, KE, X], BF16, tag="w2x", bufs=4)
            nc.sync.dma_start(
                out=w1t, in_=w1[l, si].rearrange("(k p) x -> p k x", p=128))
            w2t = wbig.tile([128, KX, E], BF16, tag="w2x", bufs=4)
            nc.sync.dma_start(
                out=w2t, in_=w2[l, si].rearrange("(k p) e -> p k e", p=128))
            b1_col = col_tile(b1f[l, si], KX, tag="ffcol", bufs=16)
            b2_col = col_tile(b2f[l, si], KE, tag="ffcol", bufs=16)
            w0_col = col_tile(cwf[l, si, 0], KX, tag="ffcol", bufs=16)
            w1c_col = col_tile(cwf[l, si, 1], KX, tag="ffcol", bufs=16)
            w2_col = col_tile(cwf[l, si, 2], KX, tag="ffcol", bufs=16)
            B_col = col_tile(bnB[l, si], KX, tag="ffcol", bufs=16)
            g_col = col_tile(lng[l, 3 if s == "b" else 4], KE, tag="lncol", bufs=16)
            bb_col = col_tile(lnb[l, 3 if s == "b" else 4], KE, tag="lncol", bufs=16)
            yield
            PHASES.append((f"ffn{l}.{s}", len(nc.inst_map)))

            hts = [None] * NC
            xts = [None] * NC
            hl0 = [None] * NC
            hf2 = [None] * NC

            fpf = [None] * NC

            def compute_h(ci):
                if fpf[ci] is None:
                    fpf[ci] = load_x_chunk(src, ci)
                if ci + 1 < NC:
                    fpf[ci + 1] = load_x_chunk(src, ci + 1)
                xt = fpf[ci]
                xts[ci] = xt
                ht = pb.tile([128, KX, C], BF16, tag="a8")
                for m in range(KX):
                    ph = ps.tile([128, C], F32, tag="mm")
                    for k in range(KE):
                        mm(ph, w1t[:, k, m * 128:(m + 1) * 128], xt[:, k, :],
                           start=(k == 0), stop=(k == KE - 1))
                    hf = pc.tile([128, C], BF16, tag="a1")
                    sc.activation(hf, ph, AF.Relu, bias=b1_col[:, m:m + 1])
                    v.tensor_scalar_min(ht[:, m, :], hf, 6.0)
                hts[ci] = ht
                l0 = phl.tile([128, KX, 1], BF16, tag="hl")
                f2 = phl.tile([128, KX, 1], BF16, tag="hf")
                for m in range(KX):
                    v.tensor_scalar_mul(l0[:, m, :], ht[:, m, C - 1:C],
                                        w0_col[:, m:m + 1])
                    v.tensor_scalar_mul(f2[:, m, :], ht[:, m, 0:1],
                                        w2_col[:, m:m + 1])
                hl0[ci], hf2[ci] = l0, f2

            def conv_elem(ci):
                # taps BN-A-folded. Stage-major in groups of 4 m so each
                # engine runs same-stage ops back-to-back; legs split
                # Act (w0 tap) / DVE (center+adds) / Pool (w2 leg).
                ht = hts[ci]
                h2 = pb.tile([128, KX, C], BF16, tag="a8")
                for m0 in range(0, KX, 4):
                    ms = range(m0, m0 + 4)
                    ta = {}
                    acc = {}
                    for m in ms:
                        ta[m] = pc.tile([128, C], BF16, tag="a1",
                                        name=f"cta{m}")
                        v.tensor_scalar_mul(ta[m][:, 0:C - 1],
                                            ht[:, m, 0:C - 1],
                                            w0_col[:, m:m + 1])
                    for m in ms:
                        acc[m] = pc.tile([128, C], BF16, tag="a1",
                                         name=f"cacc{m}")
                        v.tensor_scalar_mul(acc[m], ht[:, m, :],
                                            w1c_col[:, m:m + 1])
                    for m in ms:
                        v.tensor_tensor(out=acc[m][:, 1:C],
                                        in0=acc[m][:, 1:C],
                                        in1=ta[m][:, 0:C - 1], op=ALU.add)
                        if ci > 0:
                            v.tensor_tensor(out=acc[m][:, 0:1],
                                            in0=acc[m][:, 0:1],
                                            in1=hl0[ci - 1][:, m, :],
                                            op=ALU.add)
                    tb = {}
                    for m in ms:
                        tb[m] = pc.tile([128, C], BF16, tag="a1",
                                        name=f"ctb{m}")
                        gp.tensor_scalar_mul(tb[m][:, 1:C], ht[:, m, 1:C],
                                             w2_col[:, m:m + 1])
                    for m in ms:
                        v.tensor_tensor(out=acc[m][:, 0:C - 1],
                                        in0=acc[m][:, 0:C - 1],
                                        in1=tb[m][:, 1:C], op=ALU.add)
                    for m in ms:
                        if ci < NC - 1:
                            v.tensor_tensor(out=acc[m][:, C - 1:C],
                                            in0=acc[m][:, C - 1:C],
                                            in1=hf2[ci + 1][:, m, :],
                                            op=ALU.add)
                        a2 = pc.tile([128, C], BF16, tag="a1")
                        sc.activation(a2, acc[m], AF.Relu,
                                      bias=B_col[:, m:m + 1])
                        v.tensor_scalar_min(h2[:, m, :], a2, 6.0)
                return h2

            def conv_pw(ci, h2):
                rt = pa.tile([128, KE, C], BF16, tag="a4")
                rsq = pa.tile([128, KE, C], BF16, tag="a4")
                for m in range(KE):
                    pw = ps.tile([128, C], F32, tag="mm")
                    for k in range(KX):
                        mm(pw, w2t[:, k, m * 128:(m + 1) * 128], h2[:, k, :],
                           start=(k == 0), stop=(k == KX - 1))
                    pj = pc.tile([128, C], BF16, tag="a1")
                    sc.activation(pj, pw, AF.Identity,
                                  bias=b2_col[:, m:m + 1])
                    v.tensor_tensor(out=rt[:, m, :], in0=pj,
                                    in1=xts[ci][:, m, :], op=ALU.add)
                    v.tensor_tensor(out=rsq[:, m, :], in0=rt[:, m, :],
                                    in1=rt[:, m, :], op=ALU.mult)
                outt = pa.tile([128, KE, C], BF16, tag="a4")
                ln_apply(rt, g_col, bb_col, outt, sq=rsq)
                store_chunk(dst, ci, outt)
                hts[ci] = xts[ci] = None

            compute_h(0)
            yield "h"
            compute_h(1)
            yield "h"
            for ci in range(NC - 1):
                h2 = conv_elem(ci)
                yield "cv"
                conv_pw(ci, h2)
                yield "pw"
                if ci + 2 < NC:
                    compute_h(ci + 2)
                    yield "h"
            h2 = conv_elem(NC - 1)
            yield "cv"
            conv_pw(NC - 1, h2)
            yield "pw"

        # ---- final head (as a generator so it weaves into ffn2) ----
        def final_gen():
            fw1t = wbig.tile([128, 2 * KE, E2], BF16, tag="w", bufs=5)
            nc.sync.dma_start(out=fw1t,
                              in_=fw1.rearrange("(k p) g -> p k g", p=128))
            fw2t = wsm.tile([128, 2, E], BF16, tag="wfin", bufs=4)
            nc.sync.dma_start(out=fw2t,
                              in_=fw2.rearrange("(k p) e -> p k e", p=128))
            rw1t = wsm.tile([128, KE, E4], BF16, tag="wfin", bufs=4)
            nc.sync.dma_start(out=rw1t,
                              in_=rw1.rearrange("(k p) g -> p k g", p=128))
            rw2t = wsm.tile([128, E8], BF16, tag="wfin", bufs=4)
            nc.sync.dma_start(out=rw2t, in_=rw2)
            rw3t = wsm.tile([E8, 16], BF16, tag="wfin", bufs=4)
            nc.sync.dma_start(out=rw3t, in_=rw3p)
            rb3_row = row_tile(rb3p, 16)
            fb2_col = col_tile(fb2f, KE, tag="fcol")
            fb1_col = col_tile(fb1, 2, tag="fcol")
            flng_col = col_tile(flng, KE, tag="fcol")
            flnb_col = col_tile(flnb, KE, tag="fcol")
            rb1_col = col_tile(rb1, 1, tag="fcol")
            rb2_col = wcol.tile([E8, 1], F32, tag="fcol")
            nc.sync.dma_start(out=rb2_col, in_=rb2[:, None])
            out_ap = out_dram.ap()
            bsrc, lsrc = rs["b", (L - 1, 3)], rs["l", (L - 1, 3)]
            yield
            PHASES.append(("final", len(nc.inst_map)))

            def final_stage1(ci):
                xb = load_x_chunk(bsrc, ci)
                xl = load_x_chunk(lsrc, ci)
                f1t = pa.tile([128, 2, C], BF16, tag="a4")
                for m in range(2):
                    pf = ps.tile([128, C], F32, tag="mm")
                    for k in range(2 * KE):
                        rhs = xb[:, k, :] if k < KE else xl[:, k - KE, :]
                        mm(pf, fw1t[:, k, m * 128:(m + 1) * 128], rhs,
                           start=(k == 0), stop=(k == 2 * KE - 1))
                    f1 = pc.tile([128, C], BF16, tag="a1")
                    sc.activation(f1, pf, AF.Relu, bias=fb1_col[:, m:m + 1])
                    v.tensor_scalar_min(f1t[:, m, :], f1, 6.0)
                ft = pa.tile([128, KE, C], BF16, tag="a4")
                fsq = pa.tile([128, KE, C], BF16, tag="a4")
                for m in range(KE):
                    pf2 = ps.tile([128, C], F32, tag="mm")
                    for k in range(2):
                        mm(pf2, fw2t[:, k, m * 128:(m + 1) * 128],
                           f1t[:, k, :], start=(k == 0), stop=(k == 1))
                    sc.activation(ft[:, m, :], pf2, AF.Identity,
                                  bias=fb2_col[:, m:m + 1])
                    v.tensor_tensor(out=fsq[:, m, :], in0=ft[:, m, :],
                                    in1=ft[:, m, :], op=ALU.mult)
                frt = pa.tile([128, KE, C], BF16, tag="a4")
                ln_apply(ft, flng_col, flnb_col, frt, sq=fsq)
                # relu after LN
                frf = frt.rearrange("p k c -> p (k c)")
                v.tensor_scalar_max(frf, frf, 0.0)
                return frt

            def final_stage2(ci, frt):
                p1 = ps.tile([128, C], F32, tag="mm")
                for k in range(KE):
                    mm(p1, rw1t[:, k, :], frt[:, k, :], start=(k == 0),
                       stop=(k == KE - 1))
                h1f = pc.tile([128, C], BF16, tag="a1")
                sc.activation(h1f, p1, AF.Relu, bias=rb1_col[:, 0:1])
                h1t = pc.tile([128, C], BF16, tag="a1")
                v.tensor_scalar_min(h1t, h1f, 6.0)
                p2 = ps.tile([E8, C], F32, tag="mm")
                mm(p2, rw2t, h1t, start=True, stop=True)
                h2f = pc.tile([E8, C], BF16, tag="a1")
                sc.activation(h2f, p2, AF.Relu, bias=rb2_col[:, 0:1])
                h2t = pc.tile([E8, C], BF16, tag="a1")
                v.tensor_scalar_min(h2t, h2f, 6.0)
                ot = pc.tile([128, NTT, c.OUT], F32, tag="a1")
                for tt in range(NTT):
                    p3 = ps.tile([128, 16], F32, tag="mm")
                    mm(p3, h2t[:, tt * 128:(tt + 1) * 128], rw3t,
                       start=True, stop=False)
                    mm(p3, ONES_ROW, rb3_row, start=False, stop=True)
                    sc.activation(ot[:, tt, :], p3[:, 0:c.OUT], AF.Copy)
                nc.sync.dma_start(
                    out=out_ap[ci * C:(ci + 1) * C, :].rearrange(
                        "(tt p) o -> p tt o", p=128),
                    in_=ot)

            for ci in range(NC):
                frt = final_stage1(ci)
                yield "s1"
                final_stage2(ci, frt)
                yield "s2"

        # ---- layers (chunk-interleaved across independent streams,
        # next-layer self alphas / final head woven into the FFN) ----
        def adv(g, n=1):
            for _ in range(n):
                next(g, None)

        def make_selfs(l, prefetch):
            bsrc = rs["b", 0] if l == 0 else rs["b", (l - 1, 3)]
            lsrc = rs["l", 0] if l == 0 else rs["l", (l - 1, 3)]
            g0 = attn_gen(l, 0, bsrc, bsrc,
                          *make_self_tail(l, "b", rs["b", (l, 1)]),
                          prefetch=prefetch)
            g1 = attn_gen(l, 1, lsrc, lsrc,
                          *make_self_tail(l, "l", rs["l", (l, 1)]),
                          prefetch=prefetch)
            return g0, g1

        def weave(gb, gl, ext):
            """Dense FFN phase first (keeps PE matmul runs unbroken for
            the pstate ramp), then the ext generators (next-layer self
            alphas or final head) immediately after."""
            for _ in range(11):
                adv(gb); adv(gl)      # through cv3
            for g in ext:
                adv(g)                # ext weight loads (DMA only) overlap
            adv(gb); adv(gl)          # pw3
            for _ in range(9):
                for g in ext:
                    adv(g)

        g0, g1 = make_selfs(0, prefetch=True)
        adv(g0); adv(g1)              # wkv loads
        adv(g0); adv(g1)              # a1(0): x chunk loads + projections
        hmask_t, cmask_t = _late_consts()
        for _ in range(2 * NC - 1):   # rest of layer-0 self alphas
            adv(g0); adv(g1)
        adv(g0); adv(g1)              # bd + wq/ow loads

        for l in range(L):
            g2 = attn_gen(l, 2, rs["b", (l, 1)], rs["l", (l, 1)],
                          *make_cross_tail(l, "b", rs["b", (l, 2)]))
            g3 = attn_gen(l, 3, rs["l", (l, 1)], rs["b", (l, 1)],
                          *make_cross_tail(l, "l", rs["l", (l, 2)]))
            gb = ffn_gen(l, "b", rs["b", (l, 2)], rs["b", (l, 3)])
            gl = ffn_gen(l, "l", rs["l", (l, 2)], rs["l", (l, 3)])
            for _ in range(2):        # first self-beta steps (queue the
                adv(g0); adv(g1)      # chunk loads ahead of cross weights)
            adv(g2); adv(g3)          # cross wkv loads
            for _ in range(4 * NC - 2):
                adv(g0); adv(g1)      # rest of self betas
            for _ in range(2 * NC):   # cross alphas
                adv(g2); adv(g3)
            adv(g2); adv(g3)          # cross bd + wq/ow
            for _ in range(2):        # first cross-beta steps
                adv(g2); adv(g3)
            adv(gb); adv(gl)          # ffn weights (behind first q loads)
            for _ in range(4 * NC - 2):
                adv(g2); adv(g3)      # rest of cross betas
            if l + 1 < L:
                ng0, ng1 = make_selfs(l + 1, prefetch=True)
                weave(gb, gl, [ng0, ng1])
                g0, g1 = ng0, ng1
            else:
                weave(gb, gl, [final_gen()])

    return din, out_dram


# ======================================================================
# kernel() entry point: full inputs in, full outputs out (8-core SPMD).
# ======================================================================
import concourse.bacc as _bacc
from concourse.bass_utils import run_bass_kernel_spmd as _run_spmd

_N_CORES = 8
_CACHE = {}


def _steer_act_tables(nc):
    """Steer the act-table selection pass toward the single set that
    contains every activation function this kernel uses (ln, exp, relu,
    copy, identity, square), so one table load serves the whole program.
    The greedy pass otherwise alternates between the exp-only and ln-only
    sets, emitting ~150 LoadActFuncSet instructions (~1.3us each) that
    serialize the Act queue. Set ids stay aligned with act_info.json, so
    the emitted id remains valid for walrus."""
    from concourse.hw_specs import get_activation_tables
    AF = mybir.ActivationFunctionType
    need = {AF.Ln, AF.Exp, AF.Relu, AF.Copy, AF.Identity, AF.Square}
    try:
        tabs = get_activation_tables(nc.m.arch)
    except Exception:
        return
    best = None
    for name, funcs in tabs.items():
        if need <= funcs:
            best = name
            break
    if best is None:
        return
    for name in tabs:
        if name != best:
            tabs[name] = set()


def _get_nc():
    if "nc" not in _CACHE:
        nc = _bacc.Bacc("TRN2", target_bir_lowering=False, debug=False)
        _steer_act_tables(nc)
        build(nc, Cfg())
        nc.finalize()
        _CACHE["nc"] = nc
    return _CACHE["nc"]


def _bf16(x):
    import ml_dtypes
    return np.asarray(x, dtype=np.float32).astype(ml_dtypes.bfloat16)


def host_prep(inputs):
    """Host-side weight preprocessing: compose QKV, fold BN, convert bf16."""
    c = Cfg()
    E, X, H, L = c.E, c.X, c.H, c.L
    E4, E2, E8 = E // 4, E // 2, E // 8
    f = {k: np.asarray(v, dtype=np.float32) for k, v in inputs.items()}
    dw, uw, ub = f["dw"], f["uw"], f["ub"]
    # composed q and k|v projection weights
    wq = np.matmul(dw[:, :, 0], uw[:, :, 0])          # (L,4,E,E)
    wk = np.matmul(dw[:, :, 1], uw[:, :, 1])
    wv = np.matmul(dw[:, :, 2], uw[:, :, 2])
    wkv = np.concatenate([wk, wv], axis=-1)           # (L,4,E,2E)
    ubq = ub[:, :, 0]                                 # (L,4,E)
    ubkv = np.concatenate([ub[:, :, 1], ub[:, :, 2]], axis=-1)
    rsq = np.float32(1.0 / np.sqrt(1.0 + BN_EPS))
    A = f["bng"] * rsq                                # (L,2,X)
    B = f["cb"] * A + f["bnb"]
    # conv taps tap-major with BN scale A folded in
    cwf = f["cw"].transpose(0, 1, 3, 2) * A[:, :, None, :]  # (L,2,3,X)
    gwd = f["gw2"][:, :, 0] - f["gw2"][:, :, 1]       # (L,E4)
    # negated: kernel computes sigmoid via exp(-x + bias) with bias = -d
    gb2d = -(f["gb2"][:, 0] - f["gb2"][:, 1])[:, None]  # (L,1)
    rw3p = np.zeros((E8, 16), np.float32)
    rw3p[:, :c.OUT] = f["rw3"]
    rb3p = np.zeros((16,), np.float32)
    rb3p[:c.OUT] = f["rb3"]
    dh = E // H
    ident = np.eye(128, dtype=np.float32)
    ones = np.ones((128, 128), dtype=np.float32)
    hmask = np.zeros((E, H), dtype=np.float32)
    for ff in range(E):
        hmask[ff, ff // dh] = 1.0
    cmask = hmask.T.copy()

    b16 = dict(wq=wq, wkv=wkv, ubkv=ubkv, ow=f["ow"],
               w1=f["w1"], w2=f["w2"],
               gw1=f["gw1"], gwd=gwd, fw1=f["fw1"], fw2=f["fw2"],
               rw1=f["rw1"], rw2=f["rw2"], rw3p=rw3p,
               rb3p=rb3p, ident=ident, ones128=ones, hmask=hmask,
               cmask=cmask)
    f32 = dict(ubq=ubq, ob=f["ob"], b1=f["b1"], b2=f["b2"], fb2=f["fb2"],
               cwf=cwf, bnB=B,
               lng=f["lng"], lnb=f["lnb"],
               gb1=f["gb1"], gb2d=gb2d, fb1=f["fb1"], flng=f["flng"],
               flnb=f["flnb"], rb1=f["rb1"], rb2=f["rb2"])
    shared = {k: _bf16(v) for k, v in b16.items()}
    shared.update({k: np.ascontiguousarray(v, dtype=np.float32)
                   for k, v in f32.items()})
    return shared, f["body_feats"], f["limb_feats"]


def kernel(**inputs):
    nc = _get_nc()
    shared, body, limb = host_prep(inputs)
    in_maps = []
    for i in range(_N_CORES):
        m = dict(shared)
        m["body_feats"] = np.ascontiguousarray(_bf16(body[i]).T)
        m["limb_feats"] = np.ascontiguousarray(_bf16(limb[i]).T)
        in_maps.append(m)
    res = run_kernel_spmd_cached(nc, in_maps)
    out = np.stack([res[i]["out"] for i in range(_N_CORES)], axis=0)
    return out.astype(np.float32)


def run_kernel_spmd_cached(nc, in_maps, **kw):
    r = _run_spmd(nc, in_maps, list(range(_N_CORES)), **kw)
    _CACHE["last_result"] = r
    return r.results



# revision 93
# speedup vs baseline: 1.0250x; 1.0188x over previous
"""Dual-stream linear-attention transformer — bf16 redesign (per-core).

Layout convention (same as baseline):
  - "layout 1" activation: [E, N] feature-major; SBUF tiles [128, KE, C]
    (feature f = 128*k + p -> partition p, k-th slice; tokens on free dim).
  - alpha k/v are produced token-major per 128-token tile [128, E].
  - Residual streams live in internal DRAM as [E, N] bf16.

Key design points:
  - All matmul operands + SBUF activations bf16; biases folded into
    matmuls (rank-1 ones_row accumulates) or Act per-partition bias.
  - QKV down+up projections composed into single E x E / E x 2E weights
    on the host.
  - Every Act function stays inside ONE activation-table set
    (natural_log_exp_and_others): LayerNorm rsqrt = exp(-0.5*ln(v+eps)),
    gating sigmoid = 1/(1+exp(-x)) via DVE reciprocal, and
    _steer_act_tables biases the table-selection pass so exactly one
    LoadActFuncSet is emitted (the greedy pass otherwise thrashes
    ~84-150 reloads x 1.3us between the exp- and ln-only sets).
  - elu+1 split Act/Pool/DVE: Act Exp + Act Relu, Pool min(.,1), DVE add.
  - FFN dwconv taps are BN-A-folded on the host; per-m conv legs are
    stage-major and split Act (w0 tap via per-partition-scale Copy),
    DVE (center tap, adds, halos, relu6 min) and Pool (w2 tap mul).
  - LN tails: residual add via Act Identity(bias)+DVE TT, squares
    emitted per slice so the stats matmuls start early.
  - Scheduling is deliberately phase-DENSE (no fine interleaving): the
    TimelineSim PE pstate model runs matmuls at 2.4GHz only after ~3us
    of continuous PE activity (788ns cold / 427ns warm / 213ns hot per
    512-col bf16 matmul), so long unbroken matmul bursts beat any
    work-spreading weave. Cross-phase overlap via DRAM store->load
    roundtrips always lost: an inserted stage that is not instantly
    ready head-of-line blocks the in-order PE queue.
  - Beta-side attention weights (wq/ow) load during late alpha chunks;
    hardware limits honored: Pool has no scalar_tensor_tensor, DVE
    tensor ops may read at most one PSUM operand.
"""

from dataclasses import dataclass
from contextlib import ExitStack

import numpy as np

import concourse.bass as bass
import concourse.mybir as mybir
import concourse.tile as tile

F32 = mybir.dt.float32
BF16 = mybir.dt.bfloat16
AF = mybir.ActivationFunctionType
ALU = mybir.AluOpType

LN_EPS = 1e-5
BN_EPS = 1e-5


@dataclass
class Cfg:
    N: int = 2048
    E: int = 512
    R: int = 256
    X: int = 1024
    H: int = 8
    L: int = 3
    OUT: int = 15
    C: int = 512

    @property
    def KE(self):
        return self.E // 128

    @property
    def KX(self):
        return self.X // 128

    @property
    def NC(self):
        return self.N // self.C

    @property
    def NTT(self):
        return self.C // 128


PHASES = []


def build(nc, cfg):
    c = cfg
    E, X, H, N, C, L = c.E, c.X, c.H, c.N, c.C, c.L
    KE, KX, NC, NTT = c.KE, c.KX, c.NC, c.NTT
    E4, E2, E8 = E // 4, E // 2, E // 8

    din = {}

    def inp(name, shape, dt=BF16):
        din[name] = nc.dram_tensor(name, list(shape), dt, kind="ExternalInput")
        return din[name].ap()

    # activations (host converts to bf16 and pre-transposes to [E, N])
    body_feats = inp("body_feats", (E, N))
    limb_feats = inp("limb_feats", (E, N))
    # attention weights (host-composed)
    wq = inp("wq", (L, 4, E, E))
    wkv = inp("wkv", (L, 4, E, 2 * E))
    ubq = inp("ubq", (L, 4, E), F32)
    ubkv = inp("ubkv", (L, 4, 2 * E))
    ow = inp("ow", (L, 4, E, E))
    obf = inp("ob", (L, 4, E), F32)
    # FFN
    w1 = inp("w1", (L, 2, E, X))
    b1f = inp("b1", (L, 2, X), F32)
    cwf = inp("cwf", (L, 2, 3, X), F32)     # conv taps, tap-major, BN-A folded
    bnB = inp("bnB", (L, 2, X), F32)        # cb*A + bnb
    w2 = inp("w2", (L, 2, X, E))
    b2f = inp("b2", (L, 2, E), F32)
    lng = inp("lng", (L, 5, E), F32)
    lnb = inp("lnb", (L, 5, E), F32)
    # gating
    gw1 = inp("gw1", (L, 2 * E, E4))
    gb1 = inp("gb1", (L, E4), F32)
    gwd = inp("gwd", (L, E4))               # gw2[:,0]-gw2[:,1]
    gb2d = inp("gb2d", (L, 1), F32)         # gb2[0]-gb2[1]
    # final head
    fw1 = inp("fw1", (2 * E, E2))
    fb1 = inp("fb1", (E2,), F32)
    fw2 = inp("fw2", (E2, E))
    fb2f = inp("fb2", (E,), F32)
    flng = inp("flng", (E,), F32)
    flnb = inp("flnb", (E,), F32)
    rw1 = inp("rw1", (E, E4))
    rb1 = inp("rb1", (E4,), F32)
    rw2 = inp("rw2", (E4, E8))
    rb2 = inp("rb2", (E8,), F32)
    rw3p = inp("rw3p", (E8, 16))            # zero-padded to 16
    rb3p = inp("rb3p", (16,))               # zero-padded
    ident_in = inp("ident", (128, 128))
    ones_in = inp("ones128", (128, 128))
    hmask_in = inp("hmask", (E, H))
    cmask_in = inp("cmask", (H, E))

    out_dram = nc.dram_tensor("out", [N, c.OUT], F32, kind="ExternalOutput")

    def idram(name):
        return nc.dram_tensor(name, [E, N], BF16).ap().rearrange(
            "(k p) n -> p k n", p=128)

    rs = {}
    for s in ("b", "l"):
        for l in range(L):
            for st in (1, 2, 3):
                rs[s, (l, st)] = idram(f"r{s}_{l}_{st}")
    rs["b", 0] = body_feats.rearrange("(k p) n -> p k n", p=128)
    rs["l", 0] = limb_feats.rearrange("(k p) n -> p k n", p=128)

    lowp = nc.allow_low_precision("bf16 activations within rel-err budget")

    with tile.TileContext(nc) as tc, ExitStack() as ctx, lowp:
        p_ = ctx.enter_context
        cst = p_(tc.tile_pool(name="cst", bufs=1))
        wbig = p_(tc.tile_pool(name="wbig", bufs=3))
        wsm = p_(tc.tile_pool(name="wsm", bufs=2))
        wcol = p_(tc.tile_pool(name="wcol", bufs=10))
        wrow = p_(tc.tile_pool(name="wrow", bufs=4))
        pa = p_(tc.tile_pool(name="pa", bufs=10))      # 4KB bf16 chunk tiles
        pb = p_(tc.tile_pool(name="pb", bufs=5))      # 8KB ht tiles
        pc = p_(tc.tile_pool(name="pc", bufs=12))      # 1KB bf16 / rows
        pat = p_(tc.tile_pool(name="pat", bufs=3))    # per-attn persistents
        phl = p_(tc.tile_pool(name="phl", bufs=6))    # conv halos
        ps = p_(tc.tile_pool(name="ps", bufs=6, space="PSUM"))
        psr = p_(tc.tile_pool(name="psr", bufs=2, space="PSUM"))

        v, sc, gp = nc.vector, nc.scalar, nc.gpsimd

        def mm(out, lhsT, rhs, start, stop):
            nc.tensor.matmul(out, lhsT, rhs, start=start, stop=stop)

        # ---- constants (ones first: the alpha bias matmuls need it;
        # hmask/cmask/ident aren't read until the bd/beta stages, so their
        # loads queue behind the first weight + activation chunks) ----
        ones_t = cst.tile([128, 128], BF16, tag="ones")
        nc.sync.dma_start(out=ones_t, in_=ones_in)
        def _late_consts():
            ident_t = cst.tile([128, 128], BF16, tag="ident")
            nc.sync.dma_start(out=ident_t, in_=ident_in)
            hmask_t = cst.tile([128, KE, H], BF16, tag="hmask")
            nc.sync.dma_start(out=hmask_t,
                              in_=hmask_in.rearrange("(k p) h -> p k h",
                                                     p=128))
            cmask_t = cst.tile([H, KE, 128], BF16, tag="cmask")
            nc.sync.dma_start(out=cmask_t,
                              in_=cmask_in.rearrange("h (k p) -> h k p",
                                                     p=128))
            return hmask_t, cmask_t
        ONES_COL = ones_t[:, 0:1]
        ONES_ROW = ones_t[0:1, :]
        onesc_t = cst.tile([1, C], BF16, tag="onesc")
        v.memset(onesc_t, 1.0)
        ONES_C = onesc_t[0:1, :]
        eps_ln = cst.tile([1, 1], F32, tag="epsl")
        v.memset(eps_ln, LN_EPS)

        def col_tile(src_ap, m, tag="col", bufs=None):
            t = wcol.tile([128, m], F32, tag=tag,
                          bufs=(12 if tag == "col" else bufs))
            nc.sync.dma_start(out=t, in_=src_ap.rearrange("(m p) -> p m", p=128))
            return t

        def row_tile(src_ap, n, tag="row", pool=None):
            t = (pool or wrow).tile([1, n], BF16, tag=tag)
            nc.sync.dma_start(out=t, in_=src_ap[None, :])
            return t

        def ln_apply(xs, g_col, b_col, outt, sq=None):
            """LayerNorm over features (layout 1). xs: [128, KE, C] bf16 tile.
            outt: [128, KE, C] bf16 out. rsqrt via exp(-0.5*ln(v+eps)) keeps
            every Act func in one table set (no LoadActFuncSet thrash).
            Callers that produce xs slice-by-slice pass a precomputed sq so
            the stats matmuls start after the first slice, not the last."""
            if sq is None:
                sq = pa.tile([128, KE, C], BF16, tag="a4")
                xf = xs.rearrange("p k c -> p (k c)")
                v.tensor_tensor(out=sq.rearrange("p k c -> p (k c)"),
                                in0=xf, in1=xf, op=ALU.mult)
            ps_s = psr.tile([1, C], F32, tag="row")
            ps_ss = psr.tile([1, C], F32, tag="row")
            for m in range(KE):
                mm(ps_s, ONES_COL, xs[:, m, :], start=(m == 0),
                   stop=(m == KE - 1))
                mm(ps_ss, ONES_COL, sq[:, m, :], start=(m == 0),
                   stop=(m == KE - 1))
            mrow = pc.tile([1, C], F32, tag="row", bufs=8)
            v.tensor_scalar_mul(mrow, ps_s, 1.0 / E)
            m2 = pc.tile([1, C], F32, tag="row", bufs=8)
            v.tensor_tensor(out=m2, in0=mrow, in1=mrow, op=ALU.mult)
            var = pc.tile([1, C], F32, tag="row", bufs=8)
            v.scalar_tensor_tensor(out=var, in0=ps_ss, scalar=1.0 / E,
                                   in1=m2, op0=ALU.mult, op1=ALU.subtract)
            lnv = pc.tile([1, C], F32, tag="row", bufs=8)
            sc.activation(lnv, var, AF.Ln, bias=eps_ln[0:1, 0:1])
            srow = pc.tile([1, C], BF16, tag="rowh", bufs=6)
            sc.activation(srow, lnv, AF.Exp, scale=-0.5)
            trow = pc.tile([1, C], BF16, tag="rowh", bufs=6)
            v.scalar_tensor_tensor(out=trow, in0=ps_s, scalar=1.0 / E,
                                   in1=srow, op0=ALU.mult, op1=ALU.mult)
            sb_s = pc.tile([128, C], BF16, tag="a1")
            gp.partition_broadcast(sb_s, srow)
            sb_t = pc.tile([128, C], BF16, tag="a1")
            gp.partition_broadcast(sb_t, trow)
            for m in range(KE):
                u = pc.tile([128, C], BF16, tag="a1")
                v.tensor_tensor(out=u, in0=xs[:, m, :], in1=sb_s, op=ALU.mult)
                v.tensor_tensor(out=u, in0=u, in1=sb_t, op=ALU.subtract)
                v.tensor_scalar(out=outt[:, m, :], in0=u,
                                scalar1=g_col[:, m:m + 1],
                                scalar2=b_col[:, m:m + 1],
                                op0=ALU.mult, op1=ALU.add)

        def load_x_chunk(dram_l1, ci, tag="a4"):
            xt = pa.tile([128, KE, C], BF16, tag=tag)
            nc.sync.dma_start(out=xt, in_=dram_l1[:, :, ci * C:(ci + 1) * C])
            return xt

        def store_chunk(dram_l1, ci, t):
            gp.dma_start(out=dram_l1[:, :, ci * C:(ci + 1) * C], in_=t)


        # ---- linear attention ----
        def attn_gen(l, a, xq_dram, xkv_dram, tail_m, tail_post,
                     prefetch=True):
            """Generator stages: wkv loads | a1/a2 per chunk | bd (+ wq/ow
            loads) | b1/b2a/b2b/b3 per chunk. With prefetch=False the alpha
            x-chunk loads are issued only at their own stage, so a load
            never waits at the DMA queue head for a producer store that
            has not happened yet (used when interleaved with the producing
            FFN)."""
            wkvt = wbig.tile([128, KE, 2 * E], BF16, tag="w2x", bufs=4)
            nc.sync.dma_start(
                out=wkvt, in_=wkv[l, a].rearrange("(k p) e -> p k e", p=128))
            ubkv_row = row_tile(ubkv[l, a], 2 * E)
            yield

            PHASES.append((f"attn{l}.{a}.alpha", len(nc.inst_map)))
            kv_acc = pat.tile([128, KE, 129], F32, tag="kva", bufs=2)

            xpf = [None] * NC
            if prefetch:
                xpf[0] = load_x_chunk(xkv_dram, 0)
            k2fs = [None] * NC
            v2xs = [None] * NC

            def alpha1(ci):
                if prefetch and ci + 1 < NC:
                    xpf[ci + 1] = load_x_chunk(xkv_dram, ci + 1)
                if xpf[ci] is None:
                    xpf[ci] = load_x_chunk(xkv_dram, ci)
                xt = xpf[ci]
                xpf[ci] = None
                k2f = pa.tile([128, NTT, E], BF16, tag="a4")
                v2x = pa.tile([128, NTT, 4, 129], BF16, tag="a4")
                v.memset(v2x[:, :, :, 128:129], 1.0)
                for tt in range(NTT):
                    xs = xt[:, :, tt * 128:(tt + 1) * 128]
                    pk = ps.tile([128, E], F32, tag="mm")
                    pv = ps.tile([128, E], F32, tag="mm")
                    for k in range(KE):
                        mm(pk, xs[:, k, :], wkvt[:, k, 0:E],
                           start=(k == 0), stop=False)
                        mm(pv, xs[:, k, :], wkvt[:, k, E:2 * E],
                           start=(k == 0), stop=False)
                    mm(pk, ONES_ROW, ubkv_row[:, 0:E], start=False, stop=True)
                    mm(pv, ONES_ROW, ubkv_row[:, E:2 * E], start=False,
                       stop=True)
                    ee = pc.tile([128, E], BF16, tag="a1")
                    rr = pc.tile([128, E], BF16, tag="a1")
                    sc.activation(ee, pk, AF.Exp)
                    sc.activation(rr, pk, AF.Relu)
                    m1 = pc.tile([128, E], BF16, tag="a1")
                    gp.tensor_scalar_min(m1, ee, 1.0)
                    v.tensor_tensor(out=k2f[:, tt, :], in0=m1, in1=rr,
                                    op=ALU.add)
                    for q in range(4):
                        v.tensor_copy(v2x[:, tt, q, 0:128],
                                      pv[:, q * 128:(q + 1) * 128])
                k2fs[ci], v2xs[ci] = k2f, v2x

            def alpha2(ci):
                k2f, v2x = k2fs[ci], v2xs[ci]
                k2fs[ci] = v2xs[ci] = None
                for p in range(4):
                    pkv = ps.tile([128, 129], F32, tag="mm")
                    for tt in range(NTT):
                        mm(pkv, k2f[:, tt, p * 128:(p + 1) * 128],
                           v2x[:, tt, p, :],
                           start=(tt == 0), stop=(tt == NTT - 1))
                    if ci == 0:
                        sc.activation(kv_acc[:, p, :], pkv, AF.Copy)
                    else:
                        v.tensor_tensor(out=kv_acc[:, p, :],
                                        in0=kv_acc[:, p, :], in1=pkv,
                                        op=ALU.add)

            wqt = owt = ubq_col = ob_col = None

            def load_beta_weights():
                nonlocal wqt, owt, ubq_col, ob_col
                wqt = wbig.tile([128, KE, E], BF16, tag="w", bufs=5)
                nc.sync.dma_start(
                    out=wqt,
                    in_=wq[l, a].rearrange("(k p) e -> p k e", p=128))
                owt = wbig.tile([128, KE, E], BF16, tag="w", bufs=5)
                nc.sync.dma_start(
                    out=owt,
                    in_=ow[l, a].rearrange("(k p) e -> p k e", p=128))
                ubq_col = col_tile(ubq[l, a], KE)
                ob_col = col_tile(obf[l, a], KE)

            for ci in range(NC):
                alpha1(ci)
                yield "a1"
                alpha2(ci)
                if ci == NC - 2:
                    load_beta_weights()
                yield "a2"

            bd = pat.tile([128, KE, 128], BF16, tag="bd", bufs=2)
            v.memset(bd, 0.0)
            for p in range(4):
                v.tensor_copy(bd[0:64, p, 0:64], kv_acc[0:64, p, 0:64])
                v.tensor_copy(bd[64:128, p, 64:128],
                              kv_acc[64:128, p, 64:128])
            kmm = pat.tile([128, KE, H], BF16, tag="km")
            for k in range(KE):
                v.tensor_scalar_mul(kmm[:, k, :], hmask_t[:, k, :],
                                    kv_acc[:, k, 128:129])
            yield "bd"

            PHASES.append((f"attn{l}.{a}.beta", len(nc.inst_map)))
            qpf = [None] * NC
            qpf[0] = load_x_chunk(xq_dram, 0)
            qfs = [None] * NC
            xqs = [None] * NC

            def beta1(ci):
                if ci + 1 < NC:
                    qpf[ci + 1] = load_x_chunk(xq_dram, ci + 1)
                xq = qpf[ci]
                qpf[ci] = None
                qf = pa.tile([128, KE, C], BF16, tag="a4")
                for m in range(KE):
                    pq = ps.tile([128, C], F32, tag="mm")
                    for k in range(KE):
                        mm(pq, wqt[:, k, m * 128:(m + 1) * 128], xq[:, k, :],
                           start=(k == 0), stop=(k == KE - 1))
                    ee = pc.tile([128, C], BF16, tag="a1")
                    rr = pc.tile([128, C], BF16, tag="a1")
                    sc.activation(ee, pq, AF.Exp, bias=ubq_col[:, m:m + 1])
                    sc.activation(rr, pq, AF.Relu, bias=ubq_col[:, m:m + 1])
                    m1 = pc.tile([128, C], BF16, tag="a1")
                    gp.tensor_scalar_min(m1, ee, 1.0)
                    v.tensor_tensor(out=qf[:, m, :], in0=m1, in1=rr,
                                    op=ALU.add)
                qfs[ci], xqs[ci] = qf, xq

            for ci in range(NC):
                beta1(ci)
                yield "b1"
                qf, xq = qfs[ci], xqs[ci]
                qfs[ci] = xqs[ci] = None
                pd = psr.tile([8, C], F32, tag="row")
                for k in range(KE):
                    mm(pd, kmm[:, k, :], qf[:, k, :], start=(k == 0),
                       stop=(k == KE - 1))
                rec = pc.tile([8, C], BF16, tag="a1")
                v.reciprocal(out=rec, in_=pd)
                yield "b2a"
                att = pa.tile([128, KE, C], BF16, tag="a4")
                for m in range(KE):
                    pn = ps.tile([128, C], F32, tag="mm")
                    mm(pn, bd[:, m, :], qf[:, m, :], start=True, stop=True)
                    pr = ps.tile([128, C], F32, tag="mm")
                    mm(pr, cmask_t[:, m, :], rec, start=True, stop=True)
                    rb = pc.tile([128, C], BF16, tag="a1")
                    sc.activation(rb, pr, AF.Copy)
                    v.tensor_tensor(out=att[:, m, :], in0=pn, in1=rb,
                                    op=ALU.mult)
                yield "b2b"
                for m in range(KE):
                    pos = ps.tile([128, C], F32, tag="mm")
                    for k in range(KE):
                        mm(pos, owt[:, k, m * 128:(m + 1) * 128],
                           att[:, k, :], start=(k == 0), stop=(k == KE - 1))
                    tail_m(ci, m, pos, xq, ob_col)
                tail_post(ci, xq)
                yield "b3"

        # ---- tails ----
        def make_self_tail(l, s, dst):
            g_col = col_tile(lng[l, 0 if s == "b" else 1], KE, tag="lncol", bufs=16)
            b_col = col_tile(lnb[l, 0 if s == "b" else 1], KE, tag="lncol", bufs=16)
            rt_box = [None]

            sq_box = [None]

            def tail_m(ci, m, pos, xq, ob_col):
                if m == 0:
                    rt_box[0] = pa.tile([128, KE, C], BF16, tag="a4",
                                        name="rt")
                    sq_box[0] = pa.tile([128, KE, C], BF16, tag="a4",
                                        name="rtsq")
                pj = pc.tile([128, C], BF16, tag="a1")
                sc.activation(pj, pos, AF.Identity,
                              bias=ob_col[:, m:m + 1])
                v.tensor_tensor(out=rt_box[0][:, m, :], in0=pj,
                                in1=xq[:, m, :], op=ALU.add)
                v.tensor_tensor(out=sq_box[0][:, m, :],
                                in0=rt_box[0][:, m, :],
                                in1=rt_box[0][:, m, :], op=ALU.mult)

            def tail_post(ci, xq):
                rt = rt_box[0]
                outt = pa.tile([128, KE, C], BF16, tag="a4")
                ln_apply(rt, g_col, b_col, outt, sq=sq_box[0])
                store_chunk(dst, ci, outt)

            return tail_m, tail_post

        def make_cross_tail(l, s, dst):
            gw1t = wsm.tile([128, 2 * KE, E4], BF16, tag="ws")
            nc.sync.dma_start(out=gw1t,
                              in_=gw1[l].rearrange("(k p) g -> p k g", p=128))
            gwd_col = wcol.tile([128, 1], BF16, tag="gwd")
            nc.sync.dma_start(out=gwd_col, in_=gwd[l][:, None])
            gb1_col = col_tile(gb1[l], 1, tag="lncol", bufs=16)
            gb2d_t = pat.tile([1, 1], F32, tag="gb2d")
            nc.sync.dma_start(out=gb2d_t, in_=gb2d[l][None, :])
            g_col = col_tile(lng[l, 2], KE, tag="lncol", bufs=16)
            b_col = col_tile(lnb[l, 2], KE, tag="lncol", bufs=16)
            proj_box = [None]

            def tail_m(ci, m, pos, xq, ob_col):
                if m == 0:
                    proj_box[0] = pa.tile([128, KE, C], BF16, tag="a4",
                                          name="proj")
                sc.activation(proj_box[0][:, m, :], pos, AF.Identity,
                              bias=ob_col[:, m:m + 1])

            def tail_post(ci, xq):
                proj = proj_box[0]
                pg = ps.tile([128, C], F32, tag="mm")
                for k in range(2 * KE):
                    rhs = xq[:, k, :] if k < KE else proj[:, k - KE, :]
                    mm(pg, gw1t[:, k, :], rhs, start=(k == 0),
                       stop=(k == 2 * KE - 1))
                g1 = pc.tile([128, C], BF16, tag="a1")
                sc.activation(g1, pg, AF.Relu, bias=gb1_col[:, 0:1])
                g1t = pc.tile([128, C], BF16, tag="a1")
                v.tensor_scalar_min(g1t, g1, 6.0)
                pg2 = psr.tile([1, C], F32, tag="row")
                mm(pg2, gwd_col, g1t, start=True, stop=True)
                # sigmoid(x + d) = 1/(1 + exp(-x - d)); gb2d_t holds -d
                er = pc.tile([1, C], F32, tag="row", bufs=8)
                sc.activation(er, pg2, AF.Exp, scale=-1.0,
                              bias=gb2d_t[0:1, 0:1])
                u1 = pc.tile([1, C], F32, tag="row", bufs=8)
                v.tensor_scalar_add(u1, er, 1.0)
                bg = pc.tile([1, C], BF16, tag="rowh", bufs=6)
                v.reciprocal(out=bg, in_=u1)
                bgb = pc.tile([128, C], BF16, tag="a1")
                gp.partition_broadcast(bgb, bg)
                mt = pa.tile([128, KE, C], BF16, tag="a4")
                msq = pa.tile([128, KE, C], BF16, tag="a4")
                for m in range(KE):
                    dtmp = pc.tile([128, C], BF16, tag="a1")
                    v.tensor_tensor(out=dtmp, in0=xq[:, m, :],
                                    in1=proj[:, m, :], op=ALU.subtract)
                    v.tensor_tensor(out=dtmp, in0=dtmp, in1=bgb, op=ALU.mult)
                    v.tensor_tensor(out=mt[:, m, :], in0=dtmp,
                                    in1=proj[:, m, :], op=ALU.add)
                    v.tensor_tensor(out=msq[:, m, :], in0=mt[:, m, :],
                                    in1=mt[:, m, :], op=ALU.mult)
                outt = pa.tile([128, KE, C], BF16, tag="a4")
                ln_apply(mt, g_col, b_col, outt, sq=msq)
                store_chunk(dst, ci, outt)

            return tail_m, tail_post

        # ---- FFN ----
        def ffn_gen(l, s, src, dst):
            si = 0 if s == "b" else 1
            w1t = wbig.tile([128, KE, X], BF16, tag="w2x", bufs=4)
            nc.sync.dma_start(
                out=w1t, in_=w1[l, si].rearrange("(k p) x -> p k x", p=128))
            w2t = wbig.tile([128, KX, E], BF16, tag="w2x", bufs=4)
            nc.sync.dma_start(
                out=w2t, in_=w2[l, si].rearrange("(k p) e -> p k e", p=128))
            b1_col = col_tile(b1f[l, si], KX, tag="ffcol", bufs=16)
            b2_col = col_tile(b2f[l, si], KE, tag="ffcol", bufs=16)
            w0_col = col_tile(cwf[l, si, 0], KX, tag="ffcol", bufs=16)
            w1c_col = col_tile(cwf[l, si, 1], KX, tag="ffcol", bufs=16)
            w2_col = col_tile(cwf[l, si, 2], KX, tag="ffcol", bufs=16)
            B_col = col_tile(bnB[l, si], KX, tag="ffcol", bufs=16)
            g_col = col_tile(lng[l, 3 if s == "b" else 4], KE, tag="lncol", bufs=16)
            bb_col = col_tile(lnb[l, 3 if s == "b" else 4], KE, tag="lncol", bufs=16)
            yield
            PHASES.append((f"ffn{l}.{s}", len(nc.inst_map)))

            hts = [None] * NC
            xts = [None] * NC
            hl0 = [None] * NC
            hf2 = [None] * NC

            fpf = [None] * NC

            def compute_h(ci):
                if fpf[ci] is None:
                    fpf[ci] = load_x_chunk(src, ci)
                if ci + 1 < NC:
                    fpf[ci + 1] = load_x_chunk(src, ci + 1)
                xt = fpf[ci]
                xts[ci] = xt
                ht = pb.tile([128, KX, C], BF16, tag="a8")
                for m in range(KX):
                    ph = ps.tile([128, C], F32, tag="mm")
                    for k in range(KE):
                        mm(ph, w1t[:, k, m * 128:(m + 1) * 128], xt[:, k, :],
                           start=(k == 0), stop=(k == KE - 1))
                    hf = pc.tile([128, C], BF16, tag="a1")
                    sc.activation(hf, ph, AF.Relu, bias=b1_col[:, m:m + 1])
                    v.tensor_scalar_min(ht[:, m, :], hf, 6.0)
                hts[ci] = ht
                l0 = phl.tile([128, KX, 1], BF16, tag="hl")
                f2 = phl.tile([128, KX, 1], BF16, tag="hf")
                for m in range(KX):
                    v.tensor_scalar_mul(l0[:, m, :], ht[:, m, C - 1:C],
                                        w0_col[:, m:m + 1])
                    v.tensor_scalar_mul(f2[:, m, :], ht[:, m, 0:1],
                                        w2_col[:, m:m + 1])
                hl0[ci], hf2[ci] = l0, f2

            def conv_elem(ci):
                # taps BN-A-folded. Stage-major in groups of 4 m so each
                # engine runs same-stage ops back-to-back; legs split
                # Act (w0 tap) / DVE (center+adds) / Pool (w2 leg).
                ht = hts[ci]
                h2 = pb.tile([128, KX, C], BF16, tag="a8")
                for m0 in range(0, KX, 4):
                    ms = range(m0, m0 + 4)
                    ta = {}
                    acc = {}
                    for m in ms:
                        ta[m] = pc.tile([128, C], BF16, tag="a1",
                                        name=f"cta{m}")
                        v.tensor_scalar_mul(ta[m][:, 0:C - 1],
                                            ht[:, m, 0:C - 1],
                                            w0_col[:, m:m + 1])
                    for m in ms:
                        acc[m] = pc.tile([128, C], BF16, tag="a1",
                                         name=f"cacc{m}")
                        sc.activation(acc[m], ht[:, m, :], AF.Identity,
                                      scale=w1c_col[:, m:m + 1])
                    for m in ms:
                        v.tensor_tensor(out=acc[m][:, 1:C],
                                        in0=acc[m][:, 1:C],
                                        in1=ta[m][:, 0:C - 1], op=ALU.add)
                        if ci > 0:
                            v.tensor_tensor(out=acc[m][:, 0:1],
                                            in0=acc[m][:, 0:1],
                                            in1=hl0[ci - 1][:, m, :],
                                            op=ALU.add)
                    tb = {}
                    for m in ms:
                        tb[m] = pc.tile([128, C], BF16, tag="a1",
                                        name=f"ctb{m}")
                        gp.tensor_scalar_mul(tb[m][:, 1:C], ht[:, m, 1:C],
                                             w2_col[:, m:m + 1])
                    for m in ms:
                        v.tensor_tensor(out=acc[m][:, 0:C - 1],
                                        in0=acc[m][:, 0:C - 1],
                                        in1=tb[m][:, 1:C], op=ALU.add)
                    for m in ms:
                        if ci < NC - 1:
                            v.tensor_tensor(out=acc[m][:, C - 1:C],
                                            in0=acc[m][:, C - 1:C],
                                            in1=hf2[ci + 1][:, m, :],
                                            op=ALU.add)
                        a2 = pc.tile([128, C], BF16, tag="a1")
                        sc.activation(a2, acc[m], AF.Relu,
                                      bias=B_col[:, m:m + 1])
                        v.tensor_scalar_min(h2[:, m, :], a2, 6.0)
                return h2

            def conv_pw(ci, h2):
                rt = pa.tile([128, KE, C], BF16, tag="a4")
                rsq = pa.tile([128, KE, C], BF16, tag="a4")
                for m in range(KE):
                    pw = ps.tile([128, C], F32, tag="mm")
                    for k in range(KX):
                        mm(pw, w2t[:, k, m * 128:(m + 1) * 128], h2[:, k, :],
                           start=(k == 0), stop=(k == KX - 1))
                    pj = pc.tile([128, C], BF16, tag="a1")
                    sc.activation(pj, pw, AF.Identity,
                                  bias=b2_col[:, m:m + 1])
                    v.tensor_tensor(out=rt[:, m, :], in0=pj,
                                    in1=xts[ci][:, m, :], op=ALU.add)
                    v.tensor_tensor(out=rsq[:, m, :], in0=rt[:, m, :],
                                    in1=rt[:, m, :], op=ALU.mult)
                outt = pa.tile([128, KE, C], BF16, tag="a4")
                ln_apply(rt, g_col, bb_col, outt, sq=rsq)
                store_chunk(dst, ci, outt)
                hts[ci] = xts[ci] = None

            compute_h(0)
            yield "h"
            compute_h(1)
            yield "h"
            for ci in range(NC - 1):
                h2 = conv_elem(ci)
                yield "cv"
                conv_pw(ci, h2)
                yield "pw"
                if ci + 2 < NC:
                    compute_h(ci + 2)
                    yield "h"
            h2 = conv_elem(NC - 1)
            yield "cv"
            conv_pw(NC - 1, h2)
            yield "pw"

        # ---- final head (as a generator so it weaves into ffn2) ----
        def final_gen():
            fw1t = wbig.tile([128, 2 * KE, E2], BF16, tag="w", bufs=5)
            nc.sync.dma_start(out=fw1t,
                              in_=fw1.rearrange("(k p) g -> p k g", p=128))
            fw2t = wsm.tile([128, 2, E], BF16, tag="wfin", bufs=4)
            nc.sync.dma_start(out=fw2t,
                              in_=fw2.rearrange("(k p) e -> p k e", p=128))
            rw1t = wsm.tile([128, KE, E4], BF16, tag="wfin", bufs=4)
            nc.sync.dma_start(out=rw1t,
                              in_=rw1.rearrange("(k p) g -> p k g", p=128))
            rw2t = wsm.tile([128, E8], BF16, tag="wfin", bufs=4)
            nc.sync.dma_start(out=rw2t, in_=rw2)
            rw3t = wsm.tile([E8, 16], BF16, tag="wfin", bufs=4)
            nc.sync.dma_start(out=rw3t, in_=rw3p)
            rb3_row = row_tile(rb3p, 16)
            fb2_col = col_tile(fb2f, KE, tag="fcol")
            fb1_col = col_tile(fb1, 2, tag="fcol")
            flng_col = col_tile(flng, KE, tag="fcol")
            flnb_col = col_tile(flnb, KE, tag="fcol")
            rb1_col = col_tile(rb1, 1, tag="fcol")
            rb2_col = wcol.tile([E8, 1], F32, tag="fcol")
            nc.sync.dma_start(out=rb2_col, in_=rb2[:, None])
            out_ap = out_dram.ap()
            bsrc, lsrc = rs["b", (L - 1, 3)], rs["l", (L - 1, 3)]
            yield
            PHASES.append(("final", len(nc.inst_map)))

            def final_stage1(ci):
                xb = load_x_chunk(bsrc, ci)
                xl = load_x_chunk(lsrc, ci)
                f1t = pa.tile([128, 2, C], BF16, tag="a4")
                for m in range(2):
                    pf = ps.tile([128, C], F32, tag="mm")
                    for k in range(2 * KE):
                        rhs = xb[:, k, :] if k < KE else xl[:, k - KE, :]
                        mm(pf, fw1t[:, k, m * 128:(m + 1) * 128], rhs,
                           start=(k == 0), stop=(k == 2 * KE - 1))
                    f1 = pc.tile([128, C], BF16, tag="a1")
                    sc.activation(f1, pf, AF.Relu, bias=fb1_col[:, m:m + 1])
                    v.tensor_scalar_min(f1t[:, m, :], f1, 6.0)
                ft = pa.tile([128, KE, C], BF16, tag="a4")
                fsq = pa.tile([128, KE, C], BF16, tag="a4")
                for m in range(KE):
                    pf2 = ps.tile([128, C], F32, tag="mm")
                    for k in range(2):
                        mm(pf2, fw2t[:, k, m * 128:(m + 1) * 128],
                           f1t[:, k, :], start=(k == 0), stop=(k == 1))
                    sc.activation(ft[:, m, :], pf2, AF.Identity,
                                  bias=fb2_col[:, m:m + 1])
                    v.tensor_tensor(out=fsq[:, m, :], in0=ft[:, m, :],
                                    in1=ft[:, m, :], op=ALU.mult)
                frt = pa.tile([128, KE, C], BF16, tag="a4")
                ln_apply(ft, flng_col, flnb_col, frt, sq=fsq)
                # relu after LN
                frf = frt.rearrange("p k c -> p (k c)")
                v.tensor_scalar_max(frf, frf, 0.0)
                return frt

            def final_stage2(ci, frt):
                p1 = ps.tile([128, C], F32, tag="mm")
                for k in range(KE):
                    mm(p1, rw1t[:, k, :], frt[:, k, :], start=(k == 0),
                       stop=(k == KE - 1))
                h1f = pc.tile([128, C], BF16, tag="a1")
                sc.activation(h1f, p1, AF.Relu, bias=rb1_col[:, 0:1])
                h1t = pc.tile([128, C], BF16, tag="a1")
                v.tensor_scalar_min(h1t, h1f, 6.0)
                p2 = ps.tile([E8, C], F32, tag="mm")
                mm(p2, rw2t, h1t, start=True, stop=True)
                h2f = pc.tile([E8, C], BF16, tag="a1")
                sc.activation(h2f, p2, AF.Relu, bias=rb2_col[:, 0:1])
                h2t = pc.tile([E8, C], BF16, tag="a1")
                v.tensor_scalar_min(h2t, h2f, 6.0)
                ot = pc.tile([128, NTT, c.OUT], F32, tag="a1")
                for tt in range(NTT):
                    p3 = ps.tile([128, 16], F32, tag="mm")
                    mm(p3, h2t[:, tt * 128:(tt + 1) * 128], rw3t,
                       start=True, stop=False)
                    mm(p3, ONES_ROW, rb3_row, start=False, stop=True)
                    sc.activation(ot[:, tt, :], p3[:, 0:c.OUT], AF.Copy)
                nc.sync.dma_start(
                    out=out_ap[ci * C:(ci + 1) * C, :].rearrange(
                        "(tt p) o -> p tt o", p=128),
                    in_=ot)

            for ci in range(NC):
                frt = final_stage1(ci)
                yield "s1"
                final_stage2(ci, frt)
                yield "s2"

        # ---- layers (chunk-interleaved across independent streams,
        # next-layer self alphas / final head woven into the FFN) ----
        def adv(g, n=1):
            for _ in range(n):
                next(g, None)

        def make_selfs(l, prefetch):
            bsrc = rs["b", 0] if l == 0 else rs["b", (l - 1, 3)]
            lsrc = rs["l", 0] if l == 0 else rs["l", (l - 1, 3)]
            g0 = attn_gen(l, 0, bsrc, bsrc,
                          *make_self_tail(l, "b", rs["b", (l, 1)]),
                          prefetch=prefetch)
            g1 = attn_gen(l, 1, lsrc, lsrc,
                          *make_self_tail(l, "l", rs["l", (l, 1)]),
                          prefetch=prefetch)
            return g0, g1

        def weave(gb, gl, ext):
            """Dense FFN phase first (keeps PE matmul runs unbroken for
            the pstate ramp), then the ext generators (next-layer self
            alphas or final head) immediately after."""
            for _ in range(11):
                adv(gb); adv(gl)      # through cv3
            for g in ext:
                adv(g)                # ext weight loads (DMA only) overlap
            adv(gb); adv(gl)          # pw3
            for _ in range(9):
                for g in ext:
                    adv(g)

        g0, g1 = make_selfs(0, prefetch=True)
        adv(g0); adv(g1)              # wkv loads
        adv(g0); adv(g1)              # a1(0): x chunk loads + projections
        hmask_t, cmask_t = _late_consts()
        for _ in range(2 * NC - 1):   # rest of layer-0 self alphas
            adv(g0); adv(g1)
        adv(g0); adv(g1)              # bd + wq/ow loads

        for l in range(L):
            g2 = attn_gen(l, 2, rs["b", (l, 1)], rs["l", (l, 1)],
                          *make_cross_tail(l, "b", rs["b", (l, 2)]))
            g3 = attn_gen(l, 3, rs["l", (l, 1)], rs["b", (l, 1)],
                          *make_cross_tail(l, "l", rs["l", (l, 2)]))
            gb = ffn_gen(l, "b", rs["b", (l, 2)], rs["b", (l, 3)])
            gl = ffn_gen(l, "l", rs["l", (l, 2)], rs["l", (l, 3)])
            for _ in range(2):        # first self-beta steps (queue the
                adv(g0); adv(g1)      # chunk loads ahead of cross weights)
            adv(g2); adv(g3)          # cross wkv loads
            for _ in range(4 * NC - 2):
                adv(g0); adv(g1)      # rest of self betas
            for _ in range(2 * NC):   # cross alphas
                adv(g2); adv(g3)
            adv(g2); adv(g3)          # cross bd + wq/ow
            for _ in range(2):        # first cross-beta steps
                adv(g2); adv(g3)
            adv(gb); adv(gl)          # ffn weights (behind first q loads)
            for _ in range(4 * NC - 2):
                adv(g2); adv(g3)      # rest of cross betas
            if l + 1 < L:
                ng0, ng1 = make_selfs(l + 1, prefetch=True)
                weave(gb, gl, [ng0, ng1])
                g0, g1 = ng0, ng1
            else:
                weave(gb, gl, [final_gen()])

    return din, out_dram


# ======================================================================
# kernel() entry point: full inputs in, full outputs out (8-core SPMD).
# ======================================================================
import concourse.bacc as _bacc
from concourse.bass_utils import run_bass_kernel_spmd as _run_spmd

_N_CORES = 8
_CACHE = {}


def _steer_act_tables(nc):
    """Steer the act-table selection pass toward the single set that
    contains every activation function this kernel uses (ln, exp, relu,
    copy, identity, square), so one table load serves the whole program.
    The greedy pass otherwise alternates between the exp-only and ln-only
    sets, emitting ~150 LoadActFuncSet instructions (~1.3us each) that
    serialize the Act queue. Set ids stay aligned with act_info.json, so
    the emitted id remains valid for walrus."""
    from concourse.hw_specs import get_activation_tables
    AF = mybir.ActivationFunctionType
    need = {AF.Ln, AF.Exp, AF.Relu, AF.Copy, AF.Identity, AF.Square}
    try:
        tabs = get_activation_tables(nc.m.arch)
    except Exception:
        return
    best = None
    for name, funcs in tabs.items():
        if need <= funcs:
            best = name
            break
    if best is None:
        return
    for name in tabs:
        if name != best:
            tabs[name] = set()


def _get_nc():
    if "nc" not in _CACHE:
        nc = _bacc.Bacc("TRN2", target_bir_lowering=False, debug=False)
        _steer_act_tables(nc)
        build(nc, Cfg())
        nc.finalize()
        _CACHE["nc"] = nc
    return _CACHE["nc"]


def _bf16(x):
    import ml_dtypes
    return np.asarray(x, dtype=np.float32).astype(ml_dtypes.bfloat16)


def host_prep(inputs):
    """Host-side weight preprocessing: compose QKV, fold BN, convert bf16."""
    c = Cfg()
    E, X, H, L = c.E, c.X, c.H, c.L
    E4, E2, E8 = E // 4, E // 2, E // 8
    f = {k: np.asarray(v, dtype=np.float32) for k, v in inputs.items()}
    dw, uw, ub = f["dw"], f["uw"], f["ub"]
    # composed q and k|v projection weights
    wq = np.matmul(dw[:, :, 0], uw[:, :, 0])          # (L,4,E,E)
    wk = np.matmul(dw[:, :, 1], uw[:, :, 1])
    wv = np.matmul(dw[:, :, 2], uw[:, :, 2])
    wkv = np.concatenate([wk, wv], axis=-1)           # (L,4,E,2E)
    ubq = ub[:, :, 0]                                 # (L,4,E)
    ubkv = np.concatenate([ub[:, :, 1], ub[:, :, 2]], axis=-1)
    rsq = np.float32(1.0 / np.sqrt(1.0 + BN_EPS))
    A = f["bng"] * rsq                                # (L,2,X)
    B = f["cb"] * A + f["bnb"]
    # conv taps tap-major with BN scale A folded in
    cwf = f["cw"].transpose(0, 1, 3, 2) * A[:, :, None, :]  # (L,2,3,X)
    gwd = f["gw2"][:, :, 0] - f["gw2"][:, :, 1]       # (L,E4)
    # negated: kernel computes sigmoid via exp(-x + bias) with bias = -d
    gb2d = -(f["gb2"][:, 0] - f["gb2"][:, 1])[:, None]  # (L,1)
    rw3p = np.zeros((E8, 16), np.float32)
    rw3p[:, :c.OUT] = f["rw3"]
    rb3p = np.zeros((16,), np.float32)
    rb3p[:c.OUT] = f["rb3"]
    dh = E // H
    ident = np.eye(128, dtype=np.float32)
    ones = np.ones((128, 128), dtype=np.float32)
    hmask = np.zeros((E, H), dtype=np.float32)
    for ff in range(E):
        hmask[ff, ff // dh] = 1.0
    cmask = hmask.T.copy()

    b16 = dict(wq=wq, wkv=wkv, ubkv=ubkv, ow=f["ow"],
               w1=f["w1"], w2=f["w2"],
               gw1=f["gw1"], gwd=gwd, fw1=f["fw1"], fw2=f["fw2"],
               rw1=f["rw1"], rw2=f["rw2"], rw3p=rw3p,
               rb3p=rb3p, ident=ident, ones128=ones, hmask=hmask,
               cmask=cmask)
    f32 = dict(ubq=ubq, ob=f["ob"], b1=f["b1"], b2=f["b2"], fb2=f["fb2"],
               cwf=cwf, bnB=B,
               lng=f["lng"], lnb=f["lnb"],
               gb1=f["gb1"], gb2d=gb2d, fb1=f["fb1"], flng=f["flng"],
               flnb=f["flnb"], rb1=f["rb1"], rb2=f["rb2"])
    shared = {k: _bf16(v) for k, v in b16.items()}
    shared.update({k: np.ascontiguousarray(v, dtype=np.float32)
                   for k, v in f32.items()})
    return shared, f["body_feats"], f["limb_feats"]


def kernel(**inputs):
    nc = _get_nc()
    shared, body, limb = host_prep(inputs)
    in_maps = []
    for i in range(_N_CORES):
        m = dict(shared)
        m["body_feats"] = np.ascontiguousarray(_bf16(body[i]).T)
        m["limb_feats"] = np.ascontiguousarray(_bf16(limb[i]).T)
        in_maps.append(m)
    res = run_kernel_spmd_cached(nc, in_maps)
    out = np.stack([res[i]["out"] for i in range(_N_CORES)], axis=0)
    return out.astype(np.float32)


def run_kernel_spmd_cached(nc, in_maps, **kw):
    r = _run_spmd(nc, in_maps, list(range(_N_CORES)), **kw)
    _CACHE["last_result"] = r
    return r.results

